# revision 59
# baseline (speedup 1.0000x reference)
"""Trainium2 Bass kernel for nn_ConstraintLayer (feasibility-projection layer).

Reference computation (B=4096, IN=2048, N=512, K=1024, NQ=8):
    qm = x @ W_map.T + b_map            -> v = qm[:, :N], beta = qm[:, N]
    v_bar = v / max(||v||, 1e-12)
    kappa_lin = relu(max_j (v_bar @ D.T)_j)
    rho = v_bar @ NA_E.T
    a_i = 0.5 rho^T P_i rho ; bq_i = rho . (P_i w + q_i) ; c_i consts
    lam_i = (-bq + sqrt(bq^2 - 4 a c)) / (2a)
    kappa = max(kappa_lin, max_i 1/lam_i)
    alpha = 1/(exp(beta) + kappa)
    y = (z0 + alpha v_bar) @ NA_E.T + y1

Key structure (v2):
  * s_raw_i = v G_i v^T with G_i = NA^T P_i NA [512, 512].  Constraints
    0-3 use the Cholesky route u_i = v L_i, s_raw = |u_i|^2, drained on
    the Activation engine (Square + accumulate).  Constraints 4-7 use
    t_i = v G_i, s_raw = rowsum(t_i * v), drained on DVE
    (tensor_tensor_reduce against the bf16 v copy).  This splits the
    16K-element reduction load evenly across both drain engines.
  * All stage-3 matmuls run in fp8(e4m3) with MatmulPerfMode.DoubleRow:
    operands carry 2 contraction sub-blocks per partition, 0.5 cyc/row.
    The lower-triangular L skips its zero half via pair widths 512/256.
    Host-side scales (16x on v, power-of-2 per-constraint on L/G) keep
    fp8 out of the subnormal range; folded into CM2.
  * D == [I; -I] (asserted), so kappa_lin = max_n |v_n|: one abs-max
    reduce, done on GPSIMD (Pool).
  * 1/lam = (sqrt(disc) + bq) * (1 / -2c): no per-row division.
  * normalization folded into scalars: s = 1/(vn e^beta + kappa_raw);
    y_dev = s * (v @ NA^T); the constant +w offset is added on HOST.
  * W_map stays resident in SBUF; x streams in 4 chunked DMAs on the SP
    queue while weights load on the DVE/Act/Pool queues (no head-of-line
    blocking of stage 1).
  * Act program order: Exp first, then one table switch to the
    sqrt_and_others set which covers Square, Copy, Abs AND Sqrt --
    no table switch on the critical tail.
  * y output in bf16 (host upcasts + adds w): halves the store traffic.

Batch data-parallel over 8 cores (512 rows each).
"""

import numpy as np

import concourse.bass as bass
import concourse.mybir as mybir
import concourse.tile as tile
from concourse import bacc
from concourse.masks import make_identity

F32 = mybir.dt.float32
BF16 = mybir.dt.bfloat16
FP8 = mybir.dt.float8e4

B = 4096
IN = 2048
N = 512
K = 1024
NQ = 8
NCORES = 8
BC = B // NCORES          # 512 batch rows per core
P128 = 128
NB_IN = IN // P128        # 16
NB_N = N // P128          # 4
NB_B = BC // P128         # 4
KH = 512
NKH = K // KH             # 2
NCH = 4                   # x/W stream chunks (4 ib each)
NQH = NQ // 2             # 4 constraints per drain path

V8SCALE = 16.0            # fp8 prescale on v (folded into CM2)
FP8ON = False             # stage-3 factors in fp8 (False: bf16 fallback)
S3DT = FP8 if FP8ON else BF16

AX = mybir.AxisListType
ALU = mybir.AluOpType
ACTF = mybir.ActivationFunctionType
DR = mybir.MatmulPerfMode.DoubleRow


def _build(use_f32r=True, reps=1, timing=False, debug=False):
    del use_f32r  # kept for test.py interface compat
    nc = bacc.Bacc()
    dbg_d = {}
    if debug:
        for nm, shp in [("Dsraw", [P128, NB_B * NQ]), ("Dbq", [P128, NB_B * NQ]),
                        ("Dmdv", [P128, NB_B]), ("Dvn2", [P128, NB_B]),
                        ("Dbeta", [P128, NB_B]), ("Ds4", [P128, NB_B]),
                        ("Dvt0", [P128, BC])]:
            dbg_d[nm] = nc.dram_tensor(nm, shp, F32, kind="ExternalOutput")

    xt_d = nc.dram_tensor("XT", [IN, BC], BF16, kind="ExternalInput")
    wt_d = nc.dram_tensor("WT", [IN, N + 1], BF16, kind="ExternalInput")
    nat_d = nc.dram_tensor("NAT", [N, K], BF16, kind="ExternalInput")
    lc8_d = [
        nc.dram_tensor(f"LC8_{nb}", [P128, NQ * (nb + 1) * P128], S3DT,
                       kind="ExternalInput")
        for nb in range(NB_N)
    ]
    pwqn_d = nc.dram_tensor("PWQN", [P128, NB_N * NQ], BF16, kind="ExternalInput")
    cm2_d = nc.dram_tensor("CM2", [NB_B * NQ], F32, kind="ExternalInput")
    cinv_d = nc.dram_tensor("CINV", [NB_B * NQ], F32, kind="ExternalInput")
    if timing:
        y_d = nc.dram_tensor("Yint", [BC, K], BF16)
        yext_d = nc.dram_tensor("Y", [1, 16], F32, kind="ExternalOutput")
    else:
        y_d = nc.dram_tensor("Y", [BC, K], BF16, kind="ExternalOutput")
        yext_d = None

    with tile.TileContext(nc) as tc:
        with (
            tc.tile_pool(name="singles", bufs=1) as singles,
            tc.tile_pool(name="persist", bufs=1) as persist,
            tc.tile_pool(name="scratch", bufs=2) as scratch,
            tc.tile_pool(name="ypool", bufs=2) as ypool,
        ):
            # ---- constants / weights (loaded once, spread across queues) --
            ident = singles.tile([P128, P128], F32, name="ident")
            make_identity(nc, ident[:, :])
            # one-time loads: W first (stage 1 consumes it immediately),
            # then constraint factors and NA^T, all on the sync queue;
            # broadcast constants on gpsimd (baseline-proven placements)
            wts = []
            for ib in range(NB_IN):
                t = singles.tile([P128, N + 1], BF16, tag=f"wt{ib}",
                                 name=f"wt{ib}")
                nc.scalar.dma_start(out=t, in_=wt_d[ib * P128:(ib + 1) * P128, :])
                wts.append(t)
            lc8 = []
            for nb in range(NB_N):
                t = singles.tile([P128, NQ * (nb + 1) * P128], S3DT,
                                 tag=f"lc8_{nb}", name=f"lc8_{nb}")
                nc.scalar.dma_start(out=t, in_=lc8_d[nb][:, :])
                lc8.append(t)
            nat = []
            for nb in range(NB_N):
                t = singles.tile([P128, K], BF16, tag=f"nat{nb}", name=f"nat{nb}")
                nc.scalar.dma_start(out=t, in_=nat_d[nb * P128:(nb + 1) * P128, :])
                nat.append(t)
            pwqn = singles.tile([P128, NB_N * NQ], BF16, name="pwqn")
            nc.gpsimd.dma_start(out=pwqn, in_=pwqn_d[:, :])
            cm2 = singles.tile([P128, NB_B * NQ], F32, name="cm2")
            nc.gpsimd.dma_start(
                out=cm2, in_=bass.AP(cm2_d, 0, [[0, P128], [1, NB_B * NQ]])
            )
            cinv = singles.tile([P128, NB_B * NQ], F32, name="cinv")
            nc.gpsimd.dma_start(
                out=cinv, in_=bass.AP(cinv_d, 0, [[0, P128], [1, NB_B * NQ]])
            )

            if timing and reps > 1:
                with tc.For_i(0, reps, 1):
                    _kbody(nc, tc, persist, scratch, ypool,
                           ident, cm2, cinv, pwqn, nat,
                           lc8, xt_d, wts, y_d, dbg_d)
            else:
                for _rep in range(reps):
                    _kbody(nc, tc, persist, scratch, ypool,
                           ident, cm2, cinv, pwqn, nat,
                           lc8, xt_d, wts, y_d, dbg_d)
            if timing:
                dummy = ypool.tile([1, 16], F32, tag="dummy", name="dummy")
                nc.vector.memset(dummy, 1.0)
                nc.sync.dma_start(out=yext_d[:, :], in_=dummy)

    nc.compile()
    return nc


def _kbody(nc, tc, persist, scratch, ypool,
           ident, cm2, cinv, pwqn, nat,
           lc8, xt_d, wts, y_d, dbg_d={}):
    # ---- persistent intermediates (stable addresses across reps) ----
    vb32 = [persist.tile([P128, N], F32, tag=f"vb32_{i}", name=f"vb32_{i}")
            for i in range(NB_B)]
    vt = [persist.tile([P128, BC], BF16, tag=f"vt{i}", name=f"vt{i}")
          for i in range(NB_N)]
    u16 = persist.tile([P128, N], BF16, tag="u16", name="u16")
    u16b = persist.tile([P128, N], BF16, tag="u16b", name="u16b")
    sraw = persist.tile([P128, NB_B * NQ], F32, tag="sraw", name="sraw")
    bq32 = persist.tile([P128, NB_B * NQ], F32, tag="bq32", name="bq32")
    mdv4 = persist.tile([P128, NB_B], F32, tag="mdv4", name="mdv4")
    vn24 = persist.tile([P128, NB_B], F32, tag="vn24", name="vn24")
    beta4 = persist.tile([P128, NB_B], F32, tag="beta4", name="beta4")
    eb4 = persist.tile([P128, NB_B], F32, tag="eb4", name="eb4")
    s4 = persist.tile([P128, NB_B], F32, tag="s4", name="s4")
    vsq = persist.tile([P128, N], F32, tag="vsq", name="vsq")

    # ---- stage 1: mapper  qm[b, c] = x @ W^T  (baseline-style streaming) --
    with (
        tc.tile_pool(name="s1x", bufs=3) as s1x,
        tc.tile_pool(name="s1ps", bufs=1, space="PSUM") as s1ps,
    ):
        qm_ps = [s1ps.tile([P128, N], F32, tag=f"qm{bb}", name=f"qm{bb}", bufs=1)
                 for bb in range(NB_B)]
        # one bank per beta column: interleaved accumulation groups must not
        # share a psum zero-region (a start= zeroes the region it addresses)
        beta_ps = [s1ps.tile([P128, 1], F32, tag=f"betaps{bb}",
                             name=f"betaps{bb}", bufs=1) for bb in range(NB_B)]
        for ch in range(NCH):
            # one chunked DMA per 4 ib-blocks (alternating HWDGE queues):
            # 4x fewer DMA fixed costs than per-ib streaming.  The dram AP
            # walks (partition, ib-in-chunk, column) to match the 3D tile.
            xt_t = s1x.tile([P128, NCH, BC], BF16, tag="xt", name="xt", bufs=2)
            q = nc.sync if ch % 2 == 0 else nc.scalar
            q.dma_start(out=xt_t, in_=bass.AP(
                xt_d, ch * NCH * P128 * BC,
                [[BC, P128], [P128 * BC, NCH], [1, BC]]))
            for j in range(NCH):
                ib = ch * NCH + j
                wt_t = wts[ib]
                st = dict(start=(ib == 0), stop=(ib == NB_IN - 1))
                for bb in range(NB_B):
                    sl = xt_t[:, j, bb * P128:(bb + 1) * P128]
                    nc.tensor.matmul(qm_ps[bb][:, :], sl, wt_t[:, 0:N], **st)
                    nc.tensor.matmul(beta_ps[bb][:, :], sl, wt_t[:, N:N + 1],
                                     **st)
        # beta drains first: frees 4 banks for the transposes
        for bb in range(NB_B):
            nc.vector.tensor_copy(out=beta4[:, bb:bb + 1], in_=beta_ps[bb][:, :])
        # psum -> sbuf v copies split across Act (Copy) and DVE so the
        # transposes get fed at double rate
        for bb in range(NB_B):
            nc.vector.tensor_copy(out=vb32[bb][:, :], in_=qm_ps[bb][:, :])

    # ---- stage 2: transposes -> v^T (bf16 + scaled fp8 copies) ----
    with tc.tile_pool(name="trps", bufs=2, space="PSUM") as trps:
        for bb in range(NB_B):
            for nb in range(NB_N):
                pst = trps.tile([P128, P128], F32, tag="tr", name="tr", bufs=2)
                nc.tensor.transpose(
                    pst[:, :],
                    vb32[bb][:, nb * P128:(nb + 1) * P128],
                    ident[:, :],
                )
                nc.vector.tensor_copy(
                    out=vt[nb][:, bb * P128:(bb + 1) * P128],
                    in_=pst[:, :],
                )

    # ---- stages 3-5 share one rotating 8-bank psum pool: bq, the 32
    #      quadratic-form units, then the 8 y-chunks.  y matmuls start as
    #      soon as the oldest stage-3 banks drain -- no pool barrier. ----
    with tc.tile_pool(name="mainps", bufs=1, space="PSUM") as mainps:
        # bq first: fills the PE gap while Act produces the fp8 v^T copies
        bq_ps = mainps.tile([P128, N], F32, tag="blk", name="bqp", bufs=8)
        for bb in range(NB_B):
            for nb in range(NB_N):
                nc.tensor.matmul(
                    bq_ps[:, bb * NQ:(bb + 1) * NQ],
                    vt[nb][:, bb * P128:(bb + 1) * P128],
                    pwqn[:, nb * NQ:(nb + 1) * NQ],
                    start=(nb == 0), stop=(nb == NB_N - 1),
                )
        nc.vector.tensor_copy(out=bq32[:, :], in_=bq_ps[:, 0:NB_B * NQ])

        # --- sraw-independent per-row prep, hoisted into the stage-2/3
        #     window (Act is idle until the first drains; DVE has slack) ---
        for bb in range(NB_B):
            nc.scalar.activation(out=vsq[:, :], in_=vb32[bb][:, :],
                                 func=ACTF.Square,
                                 accum_out=vn24[:, bb:bb + 1])
        vn4 = scratch.tile([P128, NB_B], F32, tag="vn4", name="vn4", bufs=2)
        nc.scalar.activation(out=vn4[:, :], in_=vn24[:, :], func=ACTF.Sqrt)
        for bb in range(NB_B):
            nc.vector.tensor_reduce(out=mdv4[:, bb:bb + 1], in_=vb32[bb][:, :],
                                    axis=AX.X, op=ALU.max,
                                    apply_absolute_value=True)
        # exp(beta) on DVE via e^b = (e^(b/64))^64 with a cubic Taylor for
        # e^(b/64): keeps the Act engine in the sqrt table set permanently.
        xq = scratch.tile([P128, NB_B], F32, tag="xq", name="xq", bufs=2)
        nc.vector.tensor_scalar_mul(out=xq[:, :], in0=beta4[:, :],
                                    scalar1=1.0 / 64.0)
        hh = scratch.tile([P128, NB_B], F32, tag="hh", name="hh", bufs=2)
        nc.vector.tensor_scalar(out=hh[:, :], in0=xq[:, :],
                                scalar1=1.0 / 6.0, scalar2=0.5,
                                op0=ALU.mult, op1=ALU.add)
        nc.vector.tensor_mul(out=hh[:, :], in0=hh[:, :], in1=xq[:, :])
        nc.vector.tensor_scalar_add(out=hh[:, :], in0=hh[:, :], scalar1=1.0)
        nc.vector.tensor_mul(out=hh[:, :], in0=hh[:, :], in1=xq[:, :])
        nc.vector.tensor_scalar_add(out=eb4[:, :], in0=hh[:, :], scalar1=1.0)
        for _sq in range(6):
            nc.vector.tensor_mul(out=eb4[:, :], in0=eb4[:, :], in1=eb4[:, :])
        # vne = max(vn, eps) * e^beta  (the non-kappa part of 1/s)
        vne4 = scratch.tile([P128, NB_B], F32, tag="vne4", name="vne4", bufs=2)
        nc.vector.tensor_scalar_max(out=vne4[:, :], in0=vn4[:, :],
                                    scalar1=1e-12)
        nc.vector.tensor_mul(out=vne4[:, :], in0=vne4[:, :], in1=eb4[:, :])

        # stage 3: fp8 triangular quadratic forms u_i = v L_i (reversed
        # nb order skips the zero blocks); drains: 5 on Act (Square+accum
        # from psum), 3 on DVE (bf16 copy + tensor_tensor_reduce)
        N_ACT_DRAIN = 6
        for bb in range(NB_B):
            u_list = []
            for i in range(NQ):
                u_list.append(mainps.tile([P128, N], F32, tag="blk",
                                          name=f"u{bb}_{i}", bufs=8))
            # nb-outer: the stationary vt[nb] slice is reused across all 8
            # constraints so the lowering elides repeated Ldweights
            for nb in range(NB_N - 1, -1, -1):
                wcols = (nb + 1) * P128
                for i in range(NQ):
                    nc.tensor.matmul(
                        u_list[i][:, 0:wcols],
                        vt[nb][:, bb * P128:(bb + 1) * P128],
                        lc8[nb][:, i * wcols:(i + 1) * wcols],
                        start=(nb == NB_N - 1), stop=(nb == 0),
                    )
            for i in range(NQ):
                u_ps = u_list[i]
                col = sraw[:, bb * NQ + i:bb * NQ + i + 1]
                if i < N_ACT_DRAIN:
                    nc.scalar.activation(
                        out=u_ps[:, :], in_=u_ps[:, :], func=ACTF.Square,
                        accum_out=col)
                else:
                    # 3-op DVE drain from individually proven instruction
                    # types: psum->bf16 copy, square via tensor_mul,
                    # add-reduce to the sraw column
                    nc.vector.tensor_copy(out=u16[:, :], in_=u_ps[:, :])
                    nc.vector.tensor_mul(out=u16b[:, :], in0=u16[:, :],
                                         in1=u16[:, :])
                    nc.vector.tensor_reduce(out=col, in_=u16b[:, :],
                                            axis=AX.X, op=ALU.add)

            # per-block finale: s4 column ready while later blocks still
            # drain, so the y scales stream instead of piling on the tail
            bbq = slice(bb * NQ, (bb + 1) * NQ)
            t8 = scratch.tile([P128, NQ], F32, tag="t8", name="t8", bufs=2)
            nc.vector.tensor_mul(out=t8[:, :], in0=bq32[:, bbq],
                                 in1=bq32[:, bbq])
            d8 = scratch.tile([P128, NQ], F32, tag="d8", name="d8", bufs=2)
            nc.vector.tensor_mul(out=d8[:, :], in0=sraw[:, bbq],
                                 in1=cm2[:, bbq])
            nc.vector.tensor_add(out=d8[:, :], in0=d8[:, :], in1=t8[:, :])
            nc.scalar.activation(out=d8[:, :], in_=d8[:, :], func=ACTF.Sqrt)
            nc.vector.tensor_add(out=d8[:, :], in0=d8[:, :], in1=bq32[:, bbq])
            nc.vector.tensor_mul(out=d8[:, :], in0=d8[:, :], in1=cinv[:, bbq])
            kap1 = scratch.tile([P128, 1], F32, tag="kap1", name="kap1", bufs=2)
            nc.vector.tensor_reduce(out=kap1[:, :], in_=d8[:, :],
                                    axis=AX.X, op=ALU.max)
            nc.vector.tensor_max(out=kap1[:, :], in0=kap1[:, :],
                                 in1=mdv4[:, bb:bb + 1])
            nc.vector.tensor_add(out=kap1[:, :], in0=kap1[:, :],
                                 in1=vne4[:, bb:bb + 1])
            nc.vector.reciprocal(out=s4[:, bb:bb + 1], in_=kap1[:, :])


        # stage 5: y matmuls weave into the stage-3 drain shadow (same pool)
        for bb in range(NB_B):
            yt = ypool.tile([P128, K], BF16, tag="yt", name="yt", bufs=3)
            for kh in range(NKH):
                yp_t = mainps.tile([P128, N], F32, tag="blk",
                                   name=f"y{bb}_{kh}", bufs=8)
                for nb in range(NB_N):
                    nc.tensor.matmul(
                        yp_t[:, :], vt[nb][:, bb * P128:(bb + 1) * P128],
                        nat[nb][:, kh * KH:(kh + 1) * KH],
                        start=(nb == 0), stop=(nb == NB_N - 1),
                    )
                nc.vector.tensor_scalar_mul(
                    out=yt[:, kh * KH:(kh + 1) * KH], in0=yp_t[:, :],
                    scalar1=s4[:, bb:bb + 1],
                )
            yq = nc.sync if bb % 2 == 0 else nc.scalar
            yq.dma_start(out=y_d[bb * P128:(bb + 1) * P128, :], in_=yt[:, :])
        if dbg_d:
            for nm, t in [("Dsraw", sraw), ("Dbq", bq32), ("Dmdv", mdv4),
                          ("Dvn2", vn24), ("Dbeta", beta4), ("Ds4", s4)]:
                nc.sync.dma_start(out=dbg_d[nm][:, :], in_=t[:, :])
            vt0f = scratch.tile([P128, BC], F32, tag="vt0f", name="vt0f", bufs=2)
            nc.vector.tensor_copy(out=vt0f[:, :], in_=vt[0][:, :])
            nc.sync.dma_start(out=dbg_d["Dvt0"][:, :], in_=vt0f[:, :])


_NC_CACHE = {}


def _get_nc(use_f32r=True, reps=1, timing=False):
    key = (bool(use_f32r), reps, timing)
    if key not in _NC_CACHE:
        _NC_CACHE[key] = _build(use_f32r=key[0], reps=reps, timing=timing)
    return _NC_CACHE[key]


def _pow2_scale(absmax, target=200.0):
    s = 2.0 ** np.floor(np.log2(target / max(absmax, 1e-30)))
    return float(np.clip(s, 2.0 ** -10, 2.0 ** 10))


def _prepare_host(inputs):
    import ml_dtypes

    f = lambda a: np.ascontiguousarray(np.asarray(a, dtype=np.float32))
    bf = lambda a: np.ascontiguousarray(np.asarray(a).astype(ml_dtypes.bfloat16))
    f8 = lambda a: np.ascontiguousarray(
        np.asarray(a, np.float32).astype(ml_dtypes.float8_e4m3))
    x = f(inputs["x"])
    W_map = f(inputs["W_map"])
    b_map = f(inputs["b_map"])
    D = f(inputs["D"])
    NA_E = f(inputs["NA_E"])
    y1 = f(inputs["y1"])
    z0 = f(inputs["z0"])
    all_P = np.asarray(inputs["all_P"], dtype=np.float64)
    all_q = f(inputs["all_q"])
    all_r = f(inputs["all_r"])

    # structural assumptions baked into the kernel
    eye = np.eye(N, dtype=np.float32)
    assert np.allclose(D, np.concatenate([eye, -eye], axis=0), atol=1e-5), \
        "kernel assumes box constraints D == [I; -I]"
    assert np.all(b_map == 0.0), "kernel assumes zero mapper bias"

    NA64 = NA_E.astype(np.float64)
    w = (NA_E @ z0 + y1)[:, 0]                              # [K]
    w64 = w.astype(np.float64)
    Pw = all_P @ w64 + all_q[:, :, 0]                       # [NQ, K]
    cv = (
        0.5 * (all_P @ w64) @ w64
        + all_q[:, :, 0].astype(np.float64) @ w64
        + all_r[:, 0, 0]
    )                                                        # [NQ], ~ -1
    pwqn = Pw @ NA64                                         # [NQ, N]
    G = NA64.T[None, :, :] @ all_P @ NA64[None, :, :]        # [NQ, N, N]
    G = 0.5 * (G + G.transpose(0, 2, 1))

    L = np.linalg.cholesky(G)                                # lower [NQ, N, N]

    LC8 = [np.zeros((P128, NQ, (nb + 1) * P128), np.float32)
           for nb in range(NB_N)]
    svec = np.zeros(NQ)                                      # sraw_hw = svec * sraw
    v8 = V8SCALE if FP8ON else 1.0
    for i in range(NQ):
        sl = _pow2_scale(np.abs(L[i]).max())
        Ls = (L[i] * sl).astype(np.float32)
        svec[i] = (v8 * sl) ** 2
        for nb in range(NB_N):
            w_c = (nb + 1) * P128
            LC8[nb][:, i, :] = Ls[nb * P128:(nb + 1) * P128, 0:w_c]

    PWQN = np.zeros((P128, NB_N * NQ), np.float32)
    for nb in range(NB_N):
        PWQN[:, nb * NQ:(nb + 1) * NQ] = pwqn.T[nb * P128:(nb + 1) * P128, :]
    cm2 = np.tile((-2.0 * cv / svec).astype(np.float32), NB_B)   # [32]
    cinv = np.tile((1.0 / (-2.0 * cv)).astype(np.float32), NB_B)

    shared = dict(
        WT=bf(W_map.T), NAT=bf(NA_E.T),
        PWQN=bf(PWQN), CM2=f(cm2), CINV=f(cinv),
    )
    cvt8 = f8 if FP8ON else bf
    for nb in range(NB_N):
        shared[f"LC8_{nb}"] = cvt8(LC8[nb].reshape(P128, -1))
    in_maps = []
    for c in range(NCORES):
        m = dict(shared)
        m["XT"] = bf(x[c * BC:(c + 1) * BC, :].T)            # [IN, BC]
        in_maps.append(m)
    return in_maps, f(w)


def kernel(**inputs) -> np.ndarray:
    from concourse.bass_utils import run_bass_kernel_spmd

    in_maps, w = _prepare_host(inputs)
    nc = _get_nc()
    res = run_bass_kernel_spmd(nc, in_maps, core_ids=list(range(NCORES)))
    out = np.concatenate(
        [np.asarray(res.results[c]["Y"], dtype=np.float32)
         for c in range(NCORES)], axis=0)
    return out + w[None, :]


# revision 60
# speedup vs baseline: 1.0082x; 1.0082x over previous
"""Trainium2 Bass kernel for nn_ConstraintLayer (feasibility-projection layer).

Reference computation (B=4096, IN=2048, N=512, K=1024, NQ=8):
    qm = x @ W_map.T + b_map            -> v = qm[:, :N], beta = qm[:, N]
    v_bar = v / max(||v||, 1e-12)
    kappa_lin = relu(max_j (v_bar @ D.T)_j)
    rho = v_bar @ NA_E.T
    a_i = 0.5 rho^T P_i rho ; bq_i = rho . (P_i w + q_i) ; c_i consts
    lam_i = (-bq + sqrt(bq^2 - 4 a c)) / (2a)
    kappa = max(kappa_lin, max_i 1/lam_i)
    alpha = 1/(exp(beta) + kappa)
    y = (z0 + alpha v_bar) @ NA_E.T + y1

Key structure (v2):
  * s_raw_i = v G_i v^T with G_i = NA^T P_i NA [512, 512].  Constraints
    0-3 use the Cholesky route u_i = v L_i, s_raw = |u_i|^2, drained on
    the Activation engine (Square + accumulate).  Constraints 4-7 use
    t_i = v G_i, s_raw = rowsum(t_i * v), drained on DVE
    (tensor_tensor_reduce against the bf16 v copy).  This splits the
    16K-element reduction load evenly across both drain engines.
  * All stage-3 matmuls run in fp8(e4m3) with MatmulPerfMode.DoubleRow:
    operands carry 2 contraction sub-blocks per partition, 0.5 cyc/row.
    The lower-triangular L skips its zero half via pair widths 512/256.
    Host-side scales (16x on v, power-of-2 per-constraint on L/G) keep
    fp8 out of the subnormal range; folded into CM2.
  * D == [I; -I] (asserted), so kappa_lin = max_n |v_n|: one abs-max
    reduce, done on GPSIMD (Pool).
  * 1/lam = (sqrt(disc) + bq) * (1 / -2c): no per-row division.
  * normalization folded into scalars: s = 1/(vn e^beta + kappa_raw);
    y_dev = s * (v @ NA^T); the constant +w offset is added on HOST.
  * W_map stays resident in SBUF; x streams in 4 chunked DMAs on the SP
    queue while weights load on the DVE/Act/Pool queues (no head-of-line
    blocking of stage 1).
  * Act program order: Exp first, then one table switch to the
    sqrt_and_others set which covers Square, Copy, Abs AND Sqrt --
    no table switch on the critical tail.
  * y output in bf16 (host upcasts + adds w): halves the store traffic.

Batch data-parallel over 8 cores (512 rows each).
"""

import numpy as np

import concourse.bass as bass
import concourse.mybir as mybir
import concourse.tile as tile
from concourse import bacc
from concourse.masks import make_identity

F32 = mybir.dt.float32
BF16 = mybir.dt.bfloat16
FP8 = mybir.dt.float8e4

B = 4096
IN = 2048
N = 512
K = 1024
NQ = 8
NCORES = 8
BC = B // NCORES          # 512 batch rows per core
P128 = 128
NB_IN = IN // P128        # 16
NB_N = N // P128          # 4
NB_B = BC // P128         # 4
KH = 512
NKH = K // KH             # 2
NCH = 4                   # x/W stream chunks (4 ib each)
NQH = NQ // 2             # 4 constraints per drain path

V8SCALE = 16.0            # fp8 prescale on v (folded into CM2)
FP8ON = False             # stage-3 factors in fp8 (False: bf16 fallback)
S3DT = FP8 if FP8ON else BF16

AX = mybir.AxisListType
ALU = mybir.AluOpType
ACTF = mybir.ActivationFunctionType
DR = mybir.MatmulPerfMode.DoubleRow


def _build(use_f32r=True, reps=1, timing=False, debug=False):
    del use_f32r  # kept for test.py interface compat
    nc = bacc.Bacc()
    dbg_d = {}
    if debug:
        for nm, shp in [("Dsraw", [P128, NB_B * NQ]), ("Dbq", [P128, NB_B * NQ]),
                        ("Dmdv", [P128, NB_B]), ("Dvn2", [P128, NB_B]),
                        ("Dbeta", [P128, NB_B]), ("Ds4", [P128, NB_B]),
                        ("Dvt0", [P128, BC])]:
            dbg_d[nm] = nc.dram_tensor(nm, shp, F32, kind="ExternalOutput")

    xt_d = nc.dram_tensor("XT", [IN, BC], BF16, kind="ExternalInput")
    wt_d = nc.dram_tensor("WT", [IN, N + 1], BF16, kind="ExternalInput")
    nat_d = nc.dram_tensor("NAT", [N, K], BF16, kind="ExternalInput")
    lc8_d = [
        nc.dram_tensor(f"LC8_{nb}", [P128, NQ * (nb + 1) * P128], S3DT,
                       kind="ExternalInput")
        for nb in range(NB_N)
    ]
    pwqn_d = nc.dram_tensor("PWQN", [P128, NB_N * NQ], BF16, kind="ExternalInput")
    cm2_d = nc.dram_tensor("CM2", [NB_B * NQ], F32, kind="ExternalInput")
    cinv_d = nc.dram_tensor("CINV", [NB_B * NQ], F32, kind="ExternalInput")
    if timing:
        y_d = nc.dram_tensor("Yint", [BC, K], BF16)
        yext_d = nc.dram_tensor("Y", [1, 16], F32, kind="ExternalOutput")
    else:
        y_d = nc.dram_tensor("Y", [BC, K], BF16, kind="ExternalOutput")
        yext_d = None

    with tile.TileContext(nc) as tc:
        with (
            tc.tile_pool(name="singles", bufs=1) as singles,
            tc.tile_pool(name="persist", bufs=1) as persist,
            tc.tile_pool(name="scratch", bufs=2) as scratch,
            tc.tile_pool(name="ypool", bufs=2) as ypool,
        ):
            # ---- constants / weights (loaded once, spread across queues) --
            ident = singles.tile([P128, P128], F32, name="ident")
            make_identity(nc, ident[:, :])
            # one-time loads: W first (stage 1 consumes it immediately),
            # then constraint factors and NA^T, all on the sync queue;
            # broadcast constants on gpsimd (baseline-proven placements)
            wts = []
            for ib in range(NB_IN):
                t = singles.tile([P128, N + 1], BF16, tag=f"wt{ib}",
                                 name=f"wt{ib}")
                nc.scalar.dma_start(out=t, in_=wt_d[ib * P128:(ib + 1) * P128, :])
                wts.append(t)
            lc8 = []
            for nb in range(NB_N):
                t = singles.tile([P128, NQ * (nb + 1) * P128], S3DT,
                                 tag=f"lc8_{nb}", name=f"lc8_{nb}")
                nc.scalar.dma_start(out=t, in_=lc8_d[nb][:, :])
                lc8.append(t)
            nat = []
            for nb in range(NB_N):
                t = singles.tile([P128, K], BF16, tag=f"nat{nb}", name=f"nat{nb}")
                nc.scalar.dma_start(out=t, in_=nat_d[nb * P128:(nb + 1) * P128, :])
                nat.append(t)
            pwqn = singles.tile([P128, NB_N * NQ], BF16, name="pwqn")
            nc.gpsimd.dma_start(out=pwqn, in_=pwqn_d[:, :])
            cm2 = singles.tile([P128, NB_B * NQ], F32, name="cm2")
            nc.gpsimd.dma_start(
                out=cm2, in_=bass.AP(cm2_d, 0, [[0, P128], [1, NB_B * NQ]])
            )
            cinv = singles.tile([P128, NB_B * NQ], F32, name="cinv")
            nc.gpsimd.dma_start(
                out=cinv, in_=bass.AP(cinv_d, 0, [[0, P128], [1, NB_B * NQ]])
            )

            if timing and reps > 1:
                with tc.For_i(0, reps, 1):
                    _kbody(nc, tc, persist, scratch, ypool,
                           ident, cm2, cinv, pwqn, nat,
                           lc8, xt_d, wts, y_d, dbg_d)
            else:
                for _rep in range(reps):
                    _kbody(nc, tc, persist, scratch, ypool,
                           ident, cm2, cinv, pwqn, nat,
                           lc8, xt_d, wts, y_d, dbg_d)
            if timing:
                dummy = ypool.tile([1, 16], F32, tag="dummy", name="dummy")
                nc.vector.memset(dummy, 1.0)
                nc.sync.dma_start(out=yext_d[:, :], in_=dummy)

    nc.compile()
    return nc


def _kbody(nc, tc, persist, scratch, ypool,
           ident, cm2, cinv, pwqn, nat,
           lc8, xt_d, wts, y_d, dbg_d={}):
    # ---- persistent intermediates (stable addresses across reps) ----
    vb32 = [persist.tile([P128, N], F32, tag=f"vb32_{i}", name=f"vb32_{i}")
            for i in range(NB_B)]
    vt = [persist.tile([P128, BC], BF16, tag=f"vt{i}", name=f"vt{i}")
          for i in range(NB_N)]
    u16 = persist.tile([P128, N], BF16, tag="u16", name="u16")
    u16b = persist.tile([P128, N], BF16, tag="u16b", name="u16b")
    sraw = persist.tile([P128, NB_B * NQ], F32, tag="sraw", name="sraw")
    bq32 = persist.tile([P128, NB_B * NQ], F32, tag="bq32", name="bq32")
    mdv4 = persist.tile([P128, NB_B], F32, tag="mdv4", name="mdv4")
    vn24 = persist.tile([P128, NB_B], F32, tag="vn24", name="vn24")
    beta4 = persist.tile([P128, NB_B], F32, tag="beta4", name="beta4")
    eb4 = persist.tile([P128, NB_B], F32, tag="eb4", name="eb4")
    s4 = persist.tile([P128, NB_B], F32, tag="s4", name="s4")
    vsq = persist.tile([P128, N], F32, tag="vsq", name="vsq")

    # ---- stage 1: mapper  qm[b, c] = x @ W^T  (baseline-style streaming) --
    with (
        tc.tile_pool(name="s1x", bufs=3) as s1x,
        tc.tile_pool(name="s1ps", bufs=1, space="PSUM") as s1ps,
    ):
        qm_ps = [s1ps.tile([P128, N], F32, tag=f"qm{bb}", name=f"qm{bb}", bufs=1)
                 for bb in range(NB_B)]
        # one bank per beta column: interleaved accumulation groups must not
        # share a psum zero-region (a start= zeroes the region it addresses)
        beta_ps = [s1ps.tile([P128, 1], F32, tag=f"betaps{bb}",
                             name=f"betaps{bb}", bufs=1) for bb in range(NB_B)]
        for ib in range(NB_IN):
            xt_t = s1x.tile([P128, BC], BF16, tag="xt", name="xt", bufs=3)
            # alternate the x stream across the sync and Act HWDGE queues:
            # a single queue's bandwidth paces stage 1 otherwise
            q = nc.sync if ib % 2 == 0 else nc.scalar
            q.dma_start(out=xt_t, in_=xt_d[ib * P128:(ib + 1) * P128, :])
            wt_t = wts[ib]
            st = dict(start=(ib == 0), stop=(ib == NB_IN - 1))
            for bb in range(NB_B):
                sl = xt_t[:, bb * P128:(bb + 1) * P128]
                nc.tensor.matmul(qm_ps[bb][:, :], sl, wt_t[:, 0:N], **st)
                nc.tensor.matmul(beta_ps[bb][:, :], sl, wt_t[:, N:N + 1], **st)
        # beta drains first: frees 4 banks for the transposes
        for bb in range(NB_B):
            nc.vector.tensor_copy(out=beta4[:, bb:bb + 1], in_=beta_ps[bb][:, :])
        # psum -> sbuf v copies split across Act (Copy) and DVE so the
        # transposes get fed at double rate
        for bb in range(NB_B):
            if bb % 2 == 0:
                nc.scalar.activation(out=vb32[bb][:, :], in_=qm_ps[bb][:, :],
                                     func=ACTF.Copy)
            else:
                nc.vector.tensor_copy(out=vb32[bb][:, :], in_=qm_ps[bb][:, :])

    # ---- stage 2: transposes -> v^T (bf16 + scaled fp8 copies) ----
    with tc.tile_pool(name="trps", bufs=2, space="PSUM") as trps:
        for bb in range(NB_B):
            for nb in range(NB_N):
                pst = trps.tile([P128, P128], F32, tag="tr", name="tr", bufs=2)
                nc.tensor.transpose(
                    pst[:, :],
                    vb32[bb][:, nb * P128:(nb + 1) * P128],
                    ident[:, :],
                )
                nc.vector.tensor_copy(
                    out=vt[nb][:, bb * P128:(bb + 1) * P128],
                    in_=pst[:, :],
                )

    # ---- stages 3-5 share one rotating 8-bank psum pool: bq, the 32
    #      quadratic-form units, then the 8 y-chunks.  y matmuls start as
    #      soon as the oldest stage-3 banks drain -- no pool barrier. ----
    with tc.tile_pool(name="mainps", bufs=1, space="PSUM") as mainps:
        # bq first: fills the PE gap while Act produces the fp8 v^T copies
        bq_ps = mainps.tile([P128, N], F32, tag="blk", name="bqp", bufs=8)
        for bb in range(NB_B):
            for nb in range(NB_N):
                nc.tensor.matmul(
                    bq_ps[:, bb * NQ:(bb + 1) * NQ],
                    vt[nb][:, bb * P128:(bb + 1) * P128],
                    pwqn[:, nb * NQ:(nb + 1) * NQ],
                    start=(nb == 0), stop=(nb == NB_N - 1),
                )
        nc.vector.tensor_copy(out=bq32[:, :], in_=bq_ps[:, 0:NB_B * NQ])

        # --- sraw-independent per-row prep, hoisted into the stage-2/3
        #     window (Act is idle until the first drains; DVE has slack) ---
        for bb in range(NB_B):
            nc.scalar.activation(out=vsq[:, :], in_=vb32[bb][:, :],
                                 func=ACTF.Square,
                                 accum_out=vn24[:, bb:bb + 1])
        vn4 = scratch.tile([P128, NB_B], F32, tag="vn4", name="vn4", bufs=2)
        nc.scalar.activation(out=vn4[:, :], in_=vn24[:, :], func=ACTF.Sqrt)
        for bb in range(NB_B):
            nc.vector.tensor_reduce(out=mdv4[:, bb:bb + 1], in_=vb32[bb][:, :],
                                    axis=AX.X, op=ALU.max,
                                    apply_absolute_value=True)
        # exp(beta) on DVE via e^b = (e^(b/64))^64 with a cubic Taylor for
        # e^(b/64): keeps the Act engine in the sqrt table set permanently.
        xq = scratch.tile([P128, NB_B], F32, tag="xq", name="xq", bufs=2)
        nc.vector.tensor_scalar_mul(out=xq[:, :], in0=beta4[:, :],
                                    scalar1=1.0 / 64.0)
        hh = scratch.tile([P128, NB_B], F32, tag="hh", name="hh", bufs=2)
        nc.vector.tensor_scalar(out=hh[:, :], in0=xq[:, :],
                                scalar1=1.0 / 6.0, scalar2=0.5,
                                op0=ALU.mult, op1=ALU.add)
        nc.vector.tensor_mul(out=hh[:, :], in0=hh[:, :], in1=xq[:, :])
        nc.vector.tensor_scalar_add(out=hh[:, :], in0=hh[:, :], scalar1=1.0)
        nc.vector.tensor_mul(out=hh[:, :], in0=hh[:, :], in1=xq[:, :])
        nc.vector.tensor_scalar_add(out=eb4[:, :], in0=hh[:, :], scalar1=1.0)
        for _sq in range(6):
            nc.vector.tensor_mul(out=eb4[:, :], in0=eb4[:, :], in1=eb4[:, :])
        # vne = max(vn, eps) * e^beta  (the non-kappa part of 1/s)
        vne4 = scratch.tile([P128, NB_B], F32, tag="vne4", name="vne4", bufs=2)
        nc.vector.tensor_scalar_max(out=vne4[:, :], in0=vn4[:, :],
                                    scalar1=1e-12)
        nc.vector.tensor_mul(out=vne4[:, :], in0=vne4[:, :], in1=eb4[:, :])

        # stage 3: fp8 triangular quadratic forms u_i = v L_i (reversed
        # nb order skips the zero blocks); drains: 5 on Act (Square+accum
        # from psum), 3 on DVE (bf16 copy + tensor_tensor_reduce)
        N_ACT_DRAIN = 6
        for bb in range(NB_B):
            u_list = []
            for i in range(NQ):
                u_list.append(mainps.tile([P128, N], F32, tag="blk",
                                          name=f"u{bb}_{i}", bufs=8))
            # nb-outer: the stationary vt[nb] slice is reused across all 8
            # constraints so the lowering elides repeated Ldweights
            for nb in range(NB_N - 1, -1, -1):
                wcols = (nb + 1) * P128
                for i in range(NQ):
                    nc.tensor.matmul(
                        u_list[i][:, 0:wcols],
                        vt[nb][:, bb * P128:(bb + 1) * P128],
                        lc8[nb][:, i * wcols:(i + 1) * wcols],
                        start=(nb == NB_N - 1), stop=(nb == 0),
                    )
            for i in range(NQ):
                u_ps = u_list[i]
                col = sraw[:, bb * NQ + i:bb * NQ + i + 1]
                if i < N_ACT_DRAIN:
                    nc.scalar.activation(
                        out=u_ps[:, :], in_=u_ps[:, :], func=ACTF.Square,
                        accum_out=col)
                else:
                    # 3-op DVE drain from individually proven instruction
                    # types: psum->bf16 copy, square via tensor_mul,
                    # add-reduce to the sraw column
                    nc.vector.tensor_copy(out=u16[:, :], in_=u_ps[:, :])
                    nc.vector.tensor_mul(out=u16b[:, :], in0=u16[:, :],
                                         in1=u16[:, :])
                    nc.vector.tensor_reduce(out=col, in_=u16b[:, :],
                                            axis=AX.X, op=ALU.add)

            # per-block finale: s4 column ready while later blocks still
            # drain, so the y scales stream instead of piling on the tail
            bbq = slice(bb * NQ, (bb + 1) * NQ)
            t8 = scratch.tile([P128, NQ], F32, tag="t8", name="t8", bufs=2)
            nc.vector.tensor_mul(out=t8[:, :], in0=bq32[:, bbq],
                                 in1=bq32[:, bbq])
            d8 = scratch.tile([P128, NQ], F32, tag="d8", name="d8", bufs=2)
            nc.vector.tensor_mul(out=d8[:, :], in0=sraw[:, bbq],
                                 in1=cm2[:, bbq])
            nc.vector.tensor_add(out=d8[:, :], in0=d8[:, :], in1=t8[:, :])
            nc.scalar.activation(out=d8[:, :], in_=d8[:, :], func=ACTF.Sqrt)
            nc.vector.tensor_add(out=d8[:, :], in0=d8[:, :], in1=bq32[:, bbq])
            nc.vector.tensor_mul(out=d8[:, :], in0=d8[:, :], in1=cinv[:, bbq])
            kap1 = scratch.tile([P128, 1], F32, tag="kap1", name="kap1", bufs=2)
            nc.vector.tensor_reduce(out=kap1[:, :], in_=d8[:, :],
                                    axis=AX.X, op=ALU.max)
            nc.vector.tensor_max(out=kap1[:, :], in0=kap1[:, :],
                                 in1=mdv4[:, bb:bb + 1])
            nc.vector.tensor_add(out=kap1[:, :], in0=kap1[:, :],
                                 in1=vne4[:, bb:bb + 1])
            nc.vector.reciprocal(out=s4[:, bb:bb + 1], in_=kap1[:, :])


        # stage 5: y matmuls weave into the stage-3 drain shadow (same pool)
        for bb in range(NB_B):
            yt = ypool.tile([P128, K], BF16, tag="yt", name="yt", bufs=3)
            for kh in range(NKH):
                yp_t = mainps.tile([P128, N], F32, tag="blk",
                                   name=f"y{bb}_{kh}", bufs=8)
                for nb in range(NB_N):
                    nc.tensor.matmul(
                        yp_t[:, :], vt[nb][:, bb * P128:(bb + 1) * P128],
                        nat[nb][:, kh * KH:(kh + 1) * KH],
                        start=(nb == 0), stop=(nb == NB_N - 1),
                    )
                if kh == 0:
                    nc.scalar.activation(
                        out=yt[:, kh * KH:(kh + 1) * KH], in_=yp_t[:, :],
                        func=ACTF.Copy, scale=s4[:, bb:bb + 1],
                    )
                else:
                    nc.vector.tensor_scalar_mul(
                        out=yt[:, kh * KH:(kh + 1) * KH], in0=yp_t[:, :],
                        scalar1=s4[:, bb:bb + 1],
                    )
            yq = nc.sync if bb % 2 == 0 else nc.scalar
            yq.dma_start(out=y_d[bb * P128:(bb + 1) * P128, :], in_=yt[:, :])
        if dbg_d:
            for nm, t in [("Dsraw", sraw), ("Dbq", bq32), ("Dmdv", mdv4),
                          ("Dvn2", vn24), ("Dbeta", beta4), ("Ds4", s4)]:
                nc.sync.dma_start(out=dbg_d[nm][:, :], in_=t[:, :])
            vt0f = scratch.tile([P128, BC], F32, tag="vt0f", name="vt0f", bufs=2)
            nc.vector.tensor_copy(out=vt0f[:, :], in_=vt[0][:, :])
            nc.sync.dma_start(out=dbg_d["Dvt0"][:, :], in_=vt0f[:, :])


_NC_CACHE = {}


def _get_nc(use_f32r=True, reps=1, timing=False):
    key = (bool(use_f32r), reps, timing)
    if key not in _NC_CACHE:
        _NC_CACHE[key] = _build(use_f32r=key[0], reps=reps, timing=timing)
    return _NC_CACHE[key]


def _pow2_scale(absmax, target=200.0):
    s = 2.0 ** np.floor(np.log2(target / max(absmax, 1e-30)))
    return float(np.clip(s, 2.0 ** -10, 2.0 ** 10))


def _prepare_host(inputs):
    import ml_dtypes

    f = lambda a: np.ascontiguousarray(np.asarray(a, dtype=np.float32))
    bf = lambda a: np.ascontiguousarray(np.asarray(a).astype(ml_dtypes.bfloat16))
    f8 = lambda a: np.ascontiguousarray(
        np.asarray(a, np.float32).astype(ml_dtypes.float8_e4m3))
    x = f(inputs["x"])
    W_map = f(inputs["W_map"])
    b_map = f(inputs["b_map"])
    D = f(inputs["D"])
    NA_E = f(inputs["NA_E"])
    y1 = f(inputs["y1"])
    z0 = f(inputs["z0"])
    all_P = np.asarray(inputs["all_P"], dtype=np.float64)
    all_q = f(inputs["all_q"])
    all_r = f(inputs["all_r"])

    # structural assumptions baked into the kernel
    eye = np.eye(N, dtype=np.float32)
    assert np.allclose(D, np.concatenate([eye, -eye], axis=0), atol=1e-5), \
        "kernel assumes box constraints D == [I; -I]"
    assert np.all(b_map == 0.0), "kernel assumes zero mapper bias"

    NA64 = NA_E.astype(np.float64)
    w = (NA_E @ z0 + y1)[:, 0]                              # [K]
    w64 = w.astype(np.float64)
    Pw = all_P @ w64 + all_q[:, :, 0]                       # [NQ, K]
    cv = (
        0.5 * (all_P @ w64) @ w64
        + all_q[:, :, 0].astype(np.float64) @ w64
        + all_r[:, 0, 0]
    )                                                        # [NQ], ~ -1
    pwqn = Pw @ NA64                                         # [NQ, N]
    G = NA64.T[None, :, :] @ all_P @ NA64[None, :, :]        # [NQ, N, N]
    G = 0.5 * (G + G.transpose(0, 2, 1))

    L = np.linalg.cholesky(G)                                # lower [NQ, N, N]

    LC8 = [np.zeros((P128, NQ, (nb + 1) * P128), np.float32)
           for nb in range(NB_N)]
    svec = np.zeros(NQ)                                      # sraw_hw = svec * sraw
    v8 = V8SCALE if FP8ON else 1.0
    for i in range(NQ):
        sl = _pow2_scale(np.abs(L[i]).max())
        Ls = (L[i] * sl).astype(np.float32)
        svec[i] = (v8 * sl) ** 2
        for nb in range(NB_N):
            w_c = (nb + 1) * P128
            LC8[nb][:, i, :] = Ls[nb * P128:(nb + 1) * P128, 0:w_c]

    PWQN = np.zeros((P128, NB_N * NQ), np.float32)
    for nb in range(NB_N):
        PWQN[:, nb * NQ:(nb + 1) * NQ] = pwqn.T[nb * P128:(nb + 1) * P128, :]
    cm2 = np.tile((-2.0 * cv / svec).astype(np.float32), NB_B)   # [32]
    cinv = np.tile((1.0 / (-2.0 * cv)).astype(np.float32), NB_B)

    shared = dict(
        WT=bf(W_map.T), NAT=bf(NA_E.T),
        PWQN=bf(PWQN), CM2=f(cm2), CINV=f(cinv),
    )
    cvt8 = f8 if FP8ON else bf
    for nb in range(NB_N):
        shared[f"LC8_{nb}"] = cvt8(LC8[nb].reshape(P128, -1))
    in_maps = []
    for c in range(NCORES):
        m = dict(shared)
        m["XT"] = bf(x[c * BC:(c + 1) * BC, :].T)            # [IN, BC]
        in_maps.append(m)
    return in_maps, f(w)


def kernel(**inputs) -> np.ndarray:
    from concourse.bass_utils import run_bass_kernel_spmd

    in_maps, w = _prepare_host(inputs)
    nc = _get_nc()
    res = run_bass_kernel_spmd(nc, in_maps, core_ids=list(range(NCORES)))
    out = np.concatenate(
        [np.asarray(res.results[c]["Y"], dtype=np.float32)
         for c in range(NCORES)], axis=0)
    return out + w[None, :]


# revision 62
# speedup vs baseline: 1.0091x; 1.0010x over previous
"""Trainium2 Bass kernel for nn_ConstraintLayer (feasibility-projection layer).

Reference computation (B=4096, IN=2048, N=512, K=1024, NQ=8):
    qm = x @ W_map.T + b_map            -> v = qm[:, :N], beta = qm[:, N]
    v_bar = v / max(||v||, 1e-12)
    kappa_lin = relu(max_j (v_bar @ D.T)_j)
    rho = v_bar @ NA_E.T
    a_i = 0.5 rho^T P_i rho ; bq_i = rho . (P_i w + q_i) ; c_i consts
    lam_i = (-bq + sqrt(bq^2 - 4 a c)) / (2a)
    kappa = max(kappa_lin, max_i 1/lam_i)
    alpha = 1/(exp(beta) + kappa)
    y = (z0 + alpha v_bar) @ NA_E.T + y1

Key structure (v2):
  * s_raw_i = v G_i v^T with G_i = NA^T P_i NA [512, 512].  Constraints
    0-3 use the Cholesky route u_i = v L_i, s_raw = |u_i|^2, drained on
    the Activation engine (Square + accumulate).  Constraints 4-7 use
    t_i = v G_i, s_raw = rowsum(t_i * v), drained on DVE
    (tensor_tensor_reduce against the bf16 v copy).  This splits the
    16K-element reduction load evenly across both drain engines.
  * All stage-3 matmuls run in fp8(e4m3) with MatmulPerfMode.DoubleRow:
    operands carry 2 contraction sub-blocks per partition, 0.5 cyc/row.
    The lower-triangular L skips its zero half via pair widths 512/256.
    Host-side scales (16x on v, power-of-2 per-constraint on L/G) keep
    fp8 out of the subnormal range; folded into CM2.
  * D == [I; -I] (asserted), so kappa_lin = max_n |v_n|: one abs-max
    reduce, done on GPSIMD (Pool).
  * 1/lam = (sqrt(disc) + bq) * (1 / -2c): no per-row division.
  * normalization folded into scalars: s = 1/(vn e^beta + kappa_raw);
    y_dev = s * (v @ NA^T); the constant +w offset is added on HOST.
  * W_map stays resident in SBUF; x streams in 4 chunked DMAs on the SP
    queue while weights load on the DVE/Act/Pool queues (no head-of-line
    blocking of stage 1).
  * Act program order: Exp first, then one table switch to the
    sqrt_and_others set which covers Square, Copy, Abs AND Sqrt --
    no table switch on the critical tail.
  * y output in bf16 (host upcasts + adds w): halves the store traffic.

Batch data-parallel over 8 cores (512 rows each).
"""

import numpy as np

import concourse.bass as bass
import concourse.mybir as mybir
import concourse.tile as tile
from concourse import bacc
from concourse.masks import make_identity

F32 = mybir.dt.float32
BF16 = mybir.dt.bfloat16
FP8 = mybir.dt.float8e4

B = 4096
IN = 2048
N = 512
K = 1024
NQ = 8
NCORES = 8
BC = B // NCORES          # 512 batch rows per core
P128 = 128
NB_IN = IN // P128        # 16
NB_N = N // P128          # 4
NB_B = BC // P128         # 4
KH = 512
NKH = K // KH             # 2
NCH = 4                   # x/W stream chunks (4 ib each)
NQH = NQ // 2             # 4 constraints per drain path

V8SCALE = 16.0            # fp8 prescale on v (folded into CM2)
FP8ON = True              # stage-3 L factors in fp8 (moving operand only)
S3DT = FP8 if FP8ON else BF16

AX = mybir.AxisListType
ALU = mybir.AluOpType
ACTF = mybir.ActivationFunctionType
DR = mybir.MatmulPerfMode.DoubleRow


def _build(use_f32r=True, reps=1, timing=False, debug=False):
    del use_f32r  # kept for test.py interface compat
    nc = bacc.Bacc()
    dbg_d = {}
    if debug:
        for nm, shp in [("Dsraw", [P128, NB_B * NQ]), ("Dbq", [P128, NB_B * NQ]),
                        ("Dmdv", [P128, NB_B]), ("Dvn2", [P128, NB_B]),
                        ("Dbeta", [P128, NB_B]), ("Ds4", [P128, NB_B]),
                        ("Dvt0", [P128, BC])]:
            dbg_d[nm] = nc.dram_tensor(nm, shp, F32, kind="ExternalOutput")

    xt_d = nc.dram_tensor("XT", [IN, BC], BF16, kind="ExternalInput")
    wt_d = nc.dram_tensor("WT", [IN, N + 1], BF16, kind="ExternalInput")
    nat_d = nc.dram_tensor("NAT", [N, K], BF16, kind="ExternalInput")
    lc8_d = [
        nc.dram_tensor(f"LC8_{nb}", [P128, NQ * (nb + 1) * P128], S3DT,
                       kind="ExternalInput")
        for nb in range(NB_N)
    ]
    pwqn_d = nc.dram_tensor("PWQN", [P128, NB_N * NQ], BF16, kind="ExternalInput")
    cm2_d = nc.dram_tensor("CM2", [NB_B * NQ], F32, kind="ExternalInput")
    cinv_d = nc.dram_tensor("CINV", [NB_B * NQ], F32, kind="ExternalInput")
    if timing:
        y_d = nc.dram_tensor("Yint", [BC, K], BF16)
        yext_d = nc.dram_tensor("Y", [1, 16], F32, kind="ExternalOutput")
    else:
        y_d = nc.dram_tensor("Y", [BC, K], BF16, kind="ExternalOutput")
        yext_d = None

    with tile.TileContext(nc) as tc:
        with (
            tc.tile_pool(name="singles", bufs=1) as singles,
            tc.tile_pool(name="persist", bufs=1) as persist,
            tc.tile_pool(name="scratch", bufs=2) as scratch,
            tc.tile_pool(name="ypool", bufs=2) as ypool,
        ):
            # ---- constants / weights (loaded once, spread across queues) --
            ident = singles.tile([P128, P128], F32, name="ident")
            make_identity(nc, ident[:, :])
            # one-time loads: W first (stage 1 consumes it immediately),
            # then constraint factors and NA^T, all on the sync queue;
            # broadcast constants on gpsimd (baseline-proven placements)
            wts = []
            for ib in range(NB_IN):
                t = singles.tile([P128, N + 1], BF16, tag=f"wt{ib}",
                                 name=f"wt{ib}")
                nc.scalar.dma_start(out=t, in_=wt_d[ib * P128:(ib + 1) * P128, :])
                wts.append(t)
            lc8 = []
            for nb in range(NB_N):
                t = singles.tile([P128, NQ * (nb + 1) * P128], S3DT,
                                 tag=f"lc8_{nb}", name=f"lc8_{nb}")
                nc.scalar.dma_start(out=t, in_=lc8_d[nb][:, :])
                lc8.append(t)
            nat = []
            for nb in range(NB_N):
                t = singles.tile([P128, K], BF16, tag=f"nat{nb}", name=f"nat{nb}")
                nc.scalar.dma_start(out=t, in_=nat_d[nb * P128:(nb + 1) * P128, :])
                nat.append(t)
            pwqn = singles.tile([P128, NB_N * NQ], BF16, name="pwqn")
            nc.gpsimd.dma_start(out=pwqn, in_=pwqn_d[:, :])
            cm2 = singles.tile([P128, NB_B * NQ], F32, name="cm2")
            nc.gpsimd.dma_start(
                out=cm2, in_=bass.AP(cm2_d, 0, [[0, P128], [1, NB_B * NQ]])
            )
            cinv = singles.tile([P128, NB_B * NQ], F32, name="cinv")
            nc.gpsimd.dma_start(
                out=cinv, in_=bass.AP(cinv_d, 0, [[0, P128], [1, NB_B * NQ]])
            )

            if timing and reps > 1:
                with tc.For_i(0, reps, 1):
                    _kbody(nc, tc, persist, scratch, ypool,
                           ident, cm2, cinv, pwqn, nat,
                           lc8, xt_d, wts, y_d, dbg_d)
            else:
                for _rep in range(reps):
                    _kbody(nc, tc, persist, scratch, ypool,
                           ident, cm2, cinv, pwqn, nat,
                           lc8, xt_d, wts, y_d, dbg_d)
            if timing:
                dummy = ypool.tile([1, 16], F32, tag="dummy", name="dummy")
                nc.vector.memset(dummy, 1.0)
                nc.sync.dma_start(out=yext_d[:, :], in_=dummy)

    nc.compile()
    return nc


def _kbody(nc, tc, persist, scratch, ypool,
           ident, cm2, cinv, pwqn, nat,
           lc8, xt_d, wts, y_d, dbg_d={}):
    # ---- persistent intermediates (stable addresses across reps) ----
    vb32 = [persist.tile([P128, N], F32, tag=f"vb32_{i}", name=f"vb32_{i}")
            for i in range(NB_B)]
    vt = [persist.tile([P128, BC], BF16, tag=f"vt{i}", name=f"vt{i}")
          for i in range(NB_N)]
    u16 = persist.tile([P128, N], BF16, tag="u16", name="u16")
    u16b = persist.tile([P128, N], BF16, tag="u16b", name="u16b")
    sraw = persist.tile([P128, NB_B * NQ], F32, tag="sraw", name="sraw")
    bq32 = persist.tile([P128, NB_B * NQ], F32, tag="bq32", name="bq32")
    mdv4 = persist.tile([P128, NB_B], F32, tag="mdv4", name="mdv4")
    vn24 = persist.tile([P128, NB_B], F32, tag="vn24", name="vn24")
    beta4 = persist.tile([P128, NB_B], F32, tag="beta4", name="beta4")
    eb4 = persist.tile([P128, NB_B], F32, tag="eb4", name="eb4")
    s4 = persist.tile([P128, NB_B], F32, tag="s4", name="s4")
    vsq = persist.tile([P128, N], F32, tag="vsq", name="vsq")

    # ---- stage 1: mapper  qm[b, c] = x @ W^T  (baseline-style streaming) --
    with (
        tc.tile_pool(name="s1x", bufs=3) as s1x,
        tc.tile_pool(name="s1ps", bufs=1, space="PSUM") as s1ps,
    ):
        qm_ps = [s1ps.tile([P128, N], F32, tag=f"qm{bb}", name=f"qm{bb}", bufs=1)
                 for bb in range(NB_B)]
        # one bank per beta column: interleaved accumulation groups must not
        # share a psum zero-region (a start= zeroes the region it addresses)
        beta_ps = [s1ps.tile([P128, 1], F32, tag=f"betaps{bb}",
                             name=f"betaps{bb}", bufs=1) for bb in range(NB_B)]
        for ib in range(NB_IN):
            xt_t = s1x.tile([P128, BC], BF16, tag="xt", name="xt", bufs=3)
            # alternate the x stream across the sync and Act HWDGE queues:
            # a single queue's bandwidth paces stage 1 otherwise
            q = nc.sync if ib % 2 == 0 else nc.scalar
            q.dma_start(out=xt_t, in_=xt_d[ib * P128:(ib + 1) * P128, :])
            wt_t = wts[ib]
            st = dict(start=(ib == 0), stop=(ib == NB_IN - 1))
            for bb in range(NB_B):
                sl = xt_t[:, bb * P128:(bb + 1) * P128]
                nc.tensor.matmul(qm_ps[bb][:, :], sl, wt_t[:, 0:N], **st)
                nc.tensor.matmul(beta_ps[bb][:, :], sl, wt_t[:, N:N + 1], **st)
        # beta drains first: frees 4 banks for the transposes
        for bb in range(NB_B):
            nc.vector.tensor_copy(out=beta4[:, bb:bb + 1], in_=beta_ps[bb][:, :])
        # psum -> sbuf v copies split across Act (Copy) and DVE so the
        # transposes get fed at double rate
        for bb in range(NB_B):
            nc.vector.tensor_copy(out=vb32[bb][:, :], in_=qm_ps[bb][:, :])

    # ---- stage 2: transposes -> v^T (bf16 + scaled fp8 copies) ----
    with tc.tile_pool(name="trps", bufs=2, space="PSUM") as trps:
        for bb in range(NB_B):
            for nb in range(NB_N):
                pst = trps.tile([P128, P128], F32, tag="tr", name="tr", bufs=2)
                nc.tensor.transpose(
                    pst[:, :],
                    vb32[bb][:, nb * P128:(nb + 1) * P128],
                    ident[:, :],
                )
                nc.vector.tensor_copy(
                    out=vt[nb][:, bb * P128:(bb + 1) * P128],
                    in_=pst[:, :],
                )

    # ---- stages 3-5 share one rotating 8-bank psum pool: bq, the 32
    #      quadratic-form units, then the 8 y-chunks.  y matmuls start as
    #      soon as the oldest stage-3 banks drain -- no pool barrier. ----
    with tc.tile_pool(name="mainps", bufs=1, space="PSUM") as mainps:
        # bq first: fills the PE gap while Act produces the fp8 v^T copies
        bq_ps = mainps.tile([P128, N], F32, tag="blk", name="bqp", bufs=8)
        for bb in range(NB_B):
            for nb in range(NB_N):
                nc.tensor.matmul(
                    bq_ps[:, bb * NQ:(bb + 1) * NQ],
                    vt[nb][:, bb * P128:(bb + 1) * P128],
                    pwqn[:, nb * NQ:(nb + 1) * NQ],
                    start=(nb == 0), stop=(nb == NB_N - 1),
                )
        nc.vector.tensor_copy(out=bq32[:, :], in_=bq_ps[:, 0:NB_B * NQ])

        # --- sraw-independent per-row prep, hoisted into the stage-2/3
        #     window (Act is idle until the first drains; DVE has slack) ---
        for bb in range(NB_B):
            nc.scalar.activation(out=vsq[:, :], in_=vb32[bb][:, :],
                                 func=ACTF.Square,
                                 accum_out=vn24[:, bb:bb + 1])
        vn4 = scratch.tile([P128, NB_B], F32, tag="vn4", name="vn4", bufs=2)
        nc.scalar.activation(out=vn4[:, :], in_=vn24[:, :], func=ACTF.Sqrt)
        for bb in range(NB_B):
            nc.vector.tensor_reduce(out=mdv4[:, bb:bb + 1], in_=vb32[bb][:, :],
                                    axis=AX.X, op=ALU.max,
                                    apply_absolute_value=True)
        # exp(beta) on DVE via e^b = (e^(b/64))^64 with a cubic Taylor for
        # e^(b/64): keeps the Act engine in the sqrt table set permanently.
        xq = scratch.tile([P128, NB_B], F32, tag="xq", name="xq", bufs=2)
        nc.vector.tensor_scalar_mul(out=xq[:, :], in0=beta4[:, :],
                                    scalar1=1.0 / 64.0)
        hh = scratch.tile([P128, NB_B], F32, tag="hh", name="hh", bufs=2)
        nc.vector.tensor_scalar(out=hh[:, :], in0=xq[:, :],
                                scalar1=1.0 / 6.0, scalar2=0.5,
                                op0=ALU.mult, op1=ALU.add)
        nc.vector.tensor_mul(out=hh[:, :], in0=hh[:, :], in1=xq[:, :])
        nc.vector.tensor_scalar_add(out=hh[:, :], in0=hh[:, :], scalar1=1.0)
        nc.vector.tensor_mul(out=hh[:, :], in0=hh[:, :], in1=xq[:, :])
        nc.vector.tensor_scalar_add(out=eb4[:, :], in0=hh[:, :], scalar1=1.0)
        for _sq in range(6):
            nc.vector.tensor_mul(out=eb4[:, :], in0=eb4[:, :], in1=eb4[:, :])
        # vne = max(vn, eps) * e^beta  (the non-kappa part of 1/s)
        vne4 = scratch.tile([P128, NB_B], F32, tag="vne4", name="vne4", bufs=2)
        nc.vector.tensor_scalar_max(out=vne4[:, :], in0=vn4[:, :],
                                    scalar1=1e-12)
        nc.vector.tensor_mul(out=vne4[:, :], in0=vne4[:, :], in1=eb4[:, :])

        # stage 3: fp8 triangular quadratic forms u_i = v L_i (reversed
        # nb order skips the zero blocks); drains: 5 on Act (Square+accum
        # from psum), 3 on DVE (bf16 copy + tensor_tensor_reduce)
        N_ACT_DRAIN = 6
        for bb in range(NB_B):
            u_list = []
            for i in range(NQ):
                u_list.append(mainps.tile([P128, N], F32, tag="blk",
                                          name=f"u{bb}_{i}", bufs=8))
            # nb-outer: the stationary vt[nb] slice is reused across all 8
            # constraints so the lowering elides repeated Ldweights
            for nb in range(NB_N - 1, -1, -1):
                wcols = (nb + 1) * P128
                for i in range(NQ):
                    nc.tensor.matmul(
                        u_list[i][:, 0:wcols],
                        vt[nb][:, bb * P128:(bb + 1) * P128],
                        lc8[nb][:, i * wcols:(i + 1) * wcols],
                        start=(nb == NB_N - 1), stop=(nb == 0),
                    )
            for i in range(NQ):
                u_ps = u_list[i]
                col = sraw[:, bb * NQ + i:bb * NQ + i + 1]
                if i < N_ACT_DRAIN:
                    nc.scalar.activation(
                        out=u_ps[:, :], in_=u_ps[:, :], func=ACTF.Square,
                        accum_out=col)
                else:
                    # 3-op DVE drain from individually proven instruction
                    # types: psum->bf16 copy, square via tensor_mul,
                    # add-reduce to the sraw column
                    nc.vector.tensor_copy(out=u16[:, :], in_=u_ps[:, :])
                    nc.vector.tensor_mul(out=u16b[:, :], in0=u16[:, :],
                                         in1=u16[:, :])
                    nc.vector.tensor_reduce(out=col, in_=u16b[:, :],
                                            axis=AX.X, op=ALU.add)

            # per-block finale: s4 column ready while later blocks still
            # drain, so the y scales stream instead of piling on the tail
            bbq = slice(bb * NQ, (bb + 1) * NQ)
            t8 = scratch.tile([P128, NQ], F32, tag="t8", name="t8", bufs=2)
            nc.vector.tensor_mul(out=t8[:, :], in0=bq32[:, bbq],
                                 in1=bq32[:, bbq])
            d8 = scratch.tile([P128, NQ], F32, tag="d8", name="d8", bufs=2)
            nc.vector.tensor_mul(out=d8[:, :], in0=sraw[:, bbq],
                                 in1=cm2[:, bbq])
            nc.vector.tensor_add(out=d8[:, :], in0=d8[:, :], in1=t8[:, :])
            nc.scalar.activation(out=d8[:, :], in_=d8[:, :], func=ACTF.Sqrt)
            nc.vector.tensor_add(out=d8[:, :], in0=d8[:, :], in1=bq32[:, bbq])
            nc.vector.tensor_mul(out=d8[:, :], in0=d8[:, :], in1=cinv[:, bbq])
            kap1 = scratch.tile([P128, 1], F32, tag="kap1", name="kap1", bufs=2)
            nc.vector.tensor_reduce(out=kap1[:, :], in_=d8[:, :],
                                    axis=AX.X, op=ALU.max)
            nc.vector.tensor_max(out=kap1[:, :], in0=kap1[:, :],
                                 in1=mdv4[:, bb:bb + 1])
            nc.vector.tensor_add(out=kap1[:, :], in0=kap1[:, :],
                                 in1=vne4[:, bb:bb + 1])
            nc.vector.reciprocal(out=s4[:, bb:bb + 1], in_=kap1[:, :])


        # stage 5: y matmuls weave into the stage-3 drain shadow (same pool)
        for bb in range(NB_B):
            yt = ypool.tile([P128, K], BF16, tag="yt", name="yt", bufs=3)
            for kh in range(NKH):
                yp_t = mainps.tile([P128, N], F32, tag="blk",
                                   name=f"y{bb}_{kh}", bufs=8)
                for nb in range(NB_N):
                    nc.tensor.matmul(
                        yp_t[:, :], vt[nb][:, bb * P128:(bb + 1) * P128],
                        nat[nb][:, kh * KH:(kh + 1) * KH],
                        start=(nb == 0), stop=(nb == NB_N - 1),
                    )
                nc.vector.tensor_scalar_mul(
                    out=yt[:, kh * KH:(kh + 1) * KH], in0=yp_t[:, :],
                    scalar1=s4[:, bb:bb + 1],
                )
            yq = nc.sync if bb % 2 == 0 else nc.scalar
            yq.dma_start(out=y_d[bb * P128:(bb + 1) * P128, :], in_=yt[:, :])
        if dbg_d:
            for nm, t in [("Dsraw", sraw), ("Dbq", bq32), ("Dmdv", mdv4),
                          ("Dvn2", vn24), ("Dbeta", beta4), ("Ds4", s4)]:
                nc.sync.dma_start(out=dbg_d[nm][:, :], in_=t[:, :])
            vt0f = scratch.tile([P128, BC], F32, tag="vt0f", name="vt0f", bufs=2)
            nc.vector.tensor_copy(out=vt0f[:, :], in_=vt[0][:, :])
            nc.sync.dma_start(out=dbg_d["Dvt0"][:, :], in_=vt0f[:, :])


_NC_CACHE = {}


def _get_nc(use_f32r=True, reps=1, timing=False):
    key = (bool(use_f32r), reps, timing)
    if key not in _NC_CACHE:
        _NC_CACHE[key] = _build(use_f32r=key[0], reps=reps, timing=timing)
    return _NC_CACHE[key]


def _pow2_scale(absmax, target=200.0):
    s = 2.0 ** np.floor(np.log2(target / max(absmax, 1e-30)))
    return float(np.clip(s, 2.0 ** -10, 2.0 ** 10))


def _prepare_host(inputs):
    import ml_dtypes

    f = lambda a: np.ascontiguousarray(np.asarray(a, dtype=np.float32))
    bf = lambda a: np.ascontiguousarray(np.asarray(a).astype(ml_dtypes.bfloat16))
    f8 = lambda a: np.ascontiguousarray(
        np.asarray(a, np.float32).astype(ml_dtypes.float8_e4m3))
    x = f(inputs["x"])
    W_map = f(inputs["W_map"])
    b_map = f(inputs["b_map"])
    D = f(inputs["D"])
    NA_E = f(inputs["NA_E"])
    y1 = f(inputs["y1"])
    z0 = f(inputs["z0"])
    all_P = np.asarray(inputs["all_P"], dtype=np.float64)
    all_q = f(inputs["all_q"])
    all_r = f(inputs["all_r"])

    # structural assumptions baked into the kernel
    eye = np.eye(N, dtype=np.float32)
    assert np.allclose(D, np.concatenate([eye, -eye], axis=0), atol=1e-5), \
        "kernel assumes box constraints D == [I; -I]"
    assert np.all(b_map == 0.0), "kernel assumes zero mapper bias"

    NA64 = NA_E.astype(np.float64)
    w = (NA_E @ z0 + y1)[:, 0]                              # [K]
    w64 = w.astype(np.float64)
    Pw = all_P @ w64 + all_q[:, :, 0]                       # [NQ, K]
    cv = (
        0.5 * (all_P @ w64) @ w64
        + all_q[:, :, 0].astype(np.float64) @ w64
        + all_r[:, 0, 0]
    )                                                        # [NQ], ~ -1
    pwqn = Pw @ NA64                                         # [NQ, N]
    G = NA64.T[None, :, :] @ all_P @ NA64[None, :, :]        # [NQ, N, N]
    G = 0.5 * (G + G.transpose(0, 2, 1))

    L = np.linalg.cholesky(G)                                # lower [NQ, N, N]

    LC8 = [np.zeros((P128, NQ, (nb + 1) * P128), np.float32)
           for nb in range(NB_N)]
    svec = np.zeros(NQ)                                      # sraw_hw = svec * sraw
    v8 = 1.0  # stationary v stays bf16 (no device prescale)
    for i in range(NQ):
        sl = _pow2_scale(np.abs(L[i]).max())
        Ls = (L[i] * sl).astype(np.float32)
        svec[i] = (v8 * sl) ** 2
        for nb in range(NB_N):
            w_c = (nb + 1) * P128
            LC8[nb][:, i, :] = Ls[nb * P128:(nb + 1) * P128, 0:w_c]

    PWQN = np.zeros((P128, NB_N * NQ), np.float32)
    for nb in range(NB_N):
        PWQN[:, nb * NQ:(nb + 1) * NQ] = pwqn.T[nb * P128:(nb + 1) * P128, :]
    cm2 = np.tile((-2.0 * cv / svec).astype(np.float32), NB_B)   # [32]
    cinv = np.tile((1.0 / (-2.0 * cv)).astype(np.float32), NB_B)

    shared = dict(
        WT=bf(W_map.T), NAT=bf(NA_E.T),
        PWQN=bf(PWQN), CM2=f(cm2), CINV=f(cinv),
    )
    cvt8 = f8 if FP8ON else bf
    for nb in range(NB_N):
        shared[f"LC8_{nb}"] = cvt8(LC8[nb].reshape(P128, -1))
    in_maps = []
    for c in range(NCORES):
        m = dict(shared)
        m["XT"] = bf(x[c * BC:(c + 1) * BC, :].T)            # [IN, BC]
        in_maps.append(m)
    return in_maps, f(w)


def kernel(**inputs) -> np.ndarray:
    from concourse.bass_utils import run_bass_kernel_spmd

    in_maps, w = _prepare_host(inputs)
    nc = _get_nc()
    res = run_bass_kernel_spmd(nc, in_maps, core_ids=list(range(NCORES)))
    out = np.concatenate(
        [np.asarray(res.results[c]["Y"], dtype=np.float32)
         for c in range(NCORES)], axis=0)
    return out + w[None, :]


# revision 63
# speedup vs baseline: 1.0122x; 1.0031x over previous
"""Trainium2 Bass kernel for nn_ConstraintLayer (feasibility-projection layer).

Reference computation (B=4096, IN=2048, N=512, K=1024, NQ=8):
    qm = x @ W_map.T + b_map            -> v = qm[:, :N], beta = qm[:, N]
    v_bar = v / max(||v||, 1e-12)
    kappa_lin = relu(max_j (v_bar @ D.T)_j)
    rho = v_bar @ NA_E.T
    a_i = 0.5 rho^T P_i rho ; bq_i = rho . (P_i w + q_i) ; c_i consts
    lam_i = (-bq + sqrt(bq^2 - 4 a c)) / (2a)
    kappa = max(kappa_lin, max_i 1/lam_i)
    alpha = 1/(exp(beta) + kappa)
    y = (z0 + alpha v_bar) @ NA_E.T + y1

Key structure (v2):
  * s_raw_i = v G_i v^T with G_i = NA^T P_i NA [512, 512].  Constraints
    0-3 use the Cholesky route u_i = v L_i, s_raw = |u_i|^2, drained on
    the Activation engine (Square + accumulate).  Constraints 4-7 use
    t_i = v G_i, s_raw = rowsum(t_i * v), drained on DVE
    (tensor_tensor_reduce against the bf16 v copy).  This splits the
    16K-element reduction load evenly across both drain engines.
  * All stage-3 matmuls run in fp8(e4m3) with MatmulPerfMode.DoubleRow:
    operands carry 2 contraction sub-blocks per partition, 0.5 cyc/row.
    The lower-triangular L skips its zero half via pair widths 512/256.
    Host-side scales (16x on v, power-of-2 per-constraint on L/G) keep
    fp8 out of the subnormal range; folded into CM2.
  * D == [I; -I] (asserted), so kappa_lin = max_n |v_n|: one abs-max
    reduce, done on GPSIMD (Pool).
  * 1/lam = (sqrt(disc) + bq) * (1 / -2c): no per-row division.
  * normalization folded into scalars: s = 1/(vn e^beta + kappa_raw);
    y_dev = s * (v @ NA^T); the constant +w offset is added on HOST.
  * W_map stays resident in SBUF; x streams in 4 chunked DMAs on the SP
    queue while weights load on the DVE/Act/Pool queues (no head-of-line
    blocking of stage 1).
  * Act program order: Exp first, then one table switch to the
    sqrt_and_others set which covers Square, Copy, Abs AND Sqrt --
    no table switch on the critical tail.
  * y output in bf16 (host upcasts + adds w): halves the store traffic.

Batch data-parallel over 8 cores (512 rows each).
"""

import numpy as np

import concourse.bass as bass
import concourse.mybir as mybir
import concourse.tile as tile
from concourse import bacc
from concourse.masks import make_identity

F32 = mybir.dt.float32
BF16 = mybir.dt.bfloat16
FP8 = mybir.dt.float8e4

B = 4096
IN = 2048
N = 512
K = 1024
NQ = 8
NCORES = 8
BC = B // NCORES          # 512 batch rows per core
P128 = 128
NB_IN = IN // P128        # 16
NB_N = N // P128          # 4
NB_B = BC // P128         # 4
KH = 512
NKH = K // KH             # 2
NCH = 4                   # x/W stream chunks (4 ib each)
NQH = NQ // 2             # 4 constraints per drain path

V8SCALE = 16.0            # fp8 prescale on v (folded into CM2)
FP8ON = True              # stage-3 L factors in fp8 (moving operand only)
S3DT = FP8 if FP8ON else BF16

AX = mybir.AxisListType
ALU = mybir.AluOpType
ACTF = mybir.ActivationFunctionType
DR = mybir.MatmulPerfMode.DoubleRow


def _build(use_f32r=True, reps=1, timing=False, debug=False):
    del use_f32r  # kept for test.py interface compat
    nc = bacc.Bacc()
    dbg_d = {}
    if debug:
        for nm, shp in [("Dsraw", [P128, NB_B * NQ]), ("Dbq", [P128, NB_B * NQ]),
                        ("Dmdv", [P128, NB_B]), ("Dvn2", [P128, NB_B]),
                        ("Dbeta", [P128, NB_B]), ("Ds4", [P128, NB_B]),
                        ("Dvt0", [P128, BC])]:
            dbg_d[nm] = nc.dram_tensor(nm, shp, F32, kind="ExternalOutput")

    xt_d = nc.dram_tensor("XT", [IN, BC], BF16, kind="ExternalInput")
    wt_d = nc.dram_tensor("WT", [IN, N + 1], BF16, kind="ExternalInput")
    nat_d = nc.dram_tensor("NAT", [N, K], BF16, kind="ExternalInput")
    lc8_d = [
        nc.dram_tensor(f"LC8_{nb}", [P128, NQ * (nb + 1) * P128], S3DT,
                       kind="ExternalInput")
        for nb in range(NB_N)
    ]
    pwqn_d = nc.dram_tensor("PWQN", [P128, NB_N * NQ], BF16, kind="ExternalInput")
    cm2_d = nc.dram_tensor("CM2", [NB_B * NQ], F32, kind="ExternalInput")
    cinv_d = nc.dram_tensor("CINV", [NB_B * NQ], F32, kind="ExternalInput")
    if timing:
        y_d = nc.dram_tensor("Yint", [BC, K], BF16)
        yext_d = nc.dram_tensor("Y", [1, 16], F32, kind="ExternalOutput")
    else:
        y_d = nc.dram_tensor("Y", [BC, K], BF16, kind="ExternalOutput")
        yext_d = None

    with tile.TileContext(nc) as tc:
        with (
            tc.tile_pool(name="singles", bufs=1) as singles,
            tc.tile_pool(name="persist", bufs=1) as persist,
            tc.tile_pool(name="scratch", bufs=2) as scratch,
            tc.tile_pool(name="ypool", bufs=2) as ypool,
        ):
            # ---- constants / weights (loaded once, spread across queues) --
            ident = singles.tile([P128, P128], F32, name="ident")
            make_identity(nc, ident[:, :])
            # one-time loads: W first (stage 1 consumes it immediately),
            # then constraint factors and NA^T, all on the sync queue;
            # broadcast constants on gpsimd (baseline-proven placements)
            wts = []
            for ib in range(NB_IN):
                t = singles.tile([P128, N + 1], BF16, tag=f"wt{ib}",
                                 name=f"wt{ib}")
                nc.scalar.dma_start(out=t, in_=wt_d[ib * P128:(ib + 1) * P128, :])
                wts.append(t)
            lc8 = []
            for nb in range(NB_N):
                t = singles.tile([P128, NQ * (nb + 1) * P128], S3DT,
                                 tag=f"lc8_{nb}", name=f"lc8_{nb}")
                nc.scalar.dma_start(out=t, in_=lc8_d[nb][:, :])
                lc8.append(t)
            nat = []
            for nb in range(NB_N):
                t = singles.tile([P128, K], BF16, tag=f"nat{nb}", name=f"nat{nb}")
                nc.gpsimd.dma_start(out=t, in_=nat_d[nb * P128:(nb + 1) * P128, :])
                nat.append(t)
            pwqn = singles.tile([P128, NB_N * NQ], BF16, name="pwqn")
            nc.gpsimd.dma_start(out=pwqn, in_=pwqn_d[:, :])
            cm2 = singles.tile([P128, NB_B * NQ], F32, name="cm2")
            nc.gpsimd.dma_start(
                out=cm2, in_=bass.AP(cm2_d, 0, [[0, P128], [1, NB_B * NQ]])
            )
            cinv = singles.tile([P128, NB_B * NQ], F32, name="cinv")
            nc.gpsimd.dma_start(
                out=cinv, in_=bass.AP(cinv_d, 0, [[0, P128], [1, NB_B * NQ]])
            )

            if timing and reps > 1:
                with tc.For_i(0, reps, 1):
                    _kbody(nc, tc, persist, scratch, ypool,
                           ident, cm2, cinv, pwqn, nat,
                           lc8, xt_d, wts, y_d, dbg_d)
            else:
                for _rep in range(reps):
                    _kbody(nc, tc, persist, scratch, ypool,
                           ident, cm2, cinv, pwqn, nat,
                           lc8, xt_d, wts, y_d, dbg_d)
            if timing:
                dummy = ypool.tile([1, 16], F32, tag="dummy", name="dummy")
                nc.vector.memset(dummy, 1.0)
                nc.sync.dma_start(out=yext_d[:, :], in_=dummy)

    nc.compile()
    return nc


def _kbody(nc, tc, persist, scratch, ypool,
           ident, cm2, cinv, pwqn, nat,
           lc8, xt_d, wts, y_d, dbg_d={}):
    # ---- persistent intermediates (stable addresses across reps) ----
    vb32 = [persist.tile([P128, N], F32, tag=f"vb32_{i}", name=f"vb32_{i}")
            for i in range(NB_B)]
    vt = [persist.tile([P128, BC], BF16, tag=f"vt{i}", name=f"vt{i}")
          for i in range(NB_N)]
    u16 = persist.tile([P128, N], BF16, tag="u16", name="u16")
    u16b = persist.tile([P128, N], BF16, tag="u16b", name="u16b")
    sraw = persist.tile([P128, NB_B * NQ], F32, tag="sraw", name="sraw")
    bq32 = persist.tile([P128, NB_B * NQ], F32, tag="bq32", name="bq32")
    mdv4 = persist.tile([P128, NB_B], F32, tag="mdv4", name="mdv4")
    vn24 = persist.tile([P128, NB_B], F32, tag="vn24", name="vn24")
    beta4 = persist.tile([P128, NB_B], F32, tag="beta4", name="beta4")
    eb4 = persist.tile([P128, NB_B], F32, tag="eb4", name="eb4")
    s4 = persist.tile([P128, NB_B], F32, tag="s4", name="s4")
    vsq = persist.tile([P128, N], F32, tag="vsq", name="vsq")

    # ---- stage 1: mapper  qm[b, c] = x @ W^T  (baseline-style streaming) --
    with (
        tc.tile_pool(name="s1x", bufs=3) as s1x,
        tc.tile_pool(name="s1ps", bufs=1, space="PSUM") as s1ps,
    ):
        qm_ps = [s1ps.tile([P128, N], F32, tag=f"qm{bb}", name=f"qm{bb}", bufs=1)
                 for bb in range(NB_B)]
        # one bank per beta column: interleaved accumulation groups must not
        # share a psum zero-region (a start= zeroes the region it addresses)
        beta_ps = [s1ps.tile([P128, 1], F32, tag=f"betaps{bb}",
                             name=f"betaps{bb}", bufs=1) for bb in range(NB_B)]
        for ib in range(NB_IN):
            xt_t = s1x.tile([P128, BC], BF16, tag="xt", name="xt", bufs=3)
            # alternate the x stream across the sync and Act HWDGE queues:
            # a single queue's bandwidth paces stage 1 otherwise
            q = nc.sync if ib % 2 == 0 else nc.scalar
            q.dma_start(out=xt_t, in_=xt_d[ib * P128:(ib + 1) * P128, :])
            wt_t = wts[ib]
            st = dict(start=(ib == 0), stop=(ib == NB_IN - 1))
            for bb in range(NB_B):
                sl = xt_t[:, bb * P128:(bb + 1) * P128]
                nc.tensor.matmul(qm_ps[bb][:, :], sl, wt_t[:, 0:N], **st)
                nc.tensor.matmul(beta_ps[bb][:, :], sl, wt_t[:, N:N + 1], **st)
        # beta drains first: frees 4 banks for the transposes
        for bb in range(NB_B):
            nc.vector.tensor_copy(out=beta4[:, bb:bb + 1], in_=beta_ps[bb][:, :])
        # psum -> sbuf v copies split across Act (Copy) and DVE so the
        # transposes get fed at double rate
        for bb in range(NB_B):
            nc.vector.tensor_copy(out=vb32[bb][:, :], in_=qm_ps[bb][:, :])

    # ---- stage 2: transposes -> v^T (bf16 + scaled fp8 copies) ----
    with tc.tile_pool(name="trps", bufs=2, space="PSUM") as trps:
        for bb in range(NB_B):
            for nb in range(NB_N):
                pst = trps.tile([P128, P128], F32, tag="tr", name="tr", bufs=2)
                nc.tensor.transpose(
                    pst[:, :],
                    vb32[bb][:, nb * P128:(nb + 1) * P128],
                    ident[:, :],
                )
                nc.vector.tensor_copy(
                    out=vt[nb][:, bb * P128:(bb + 1) * P128],
                    in_=pst[:, :],
                )

    # ---- stages 3-5 share one rotating 8-bank psum pool: bq, the 32
    #      quadratic-form units, then the 8 y-chunks.  y matmuls start as
    #      soon as the oldest stage-3 banks drain -- no pool barrier. ----
    with tc.tile_pool(name="mainps", bufs=1, space="PSUM") as mainps:
        # bq first: fills the PE gap while Act produces the fp8 v^T copies
        bq_ps = mainps.tile([P128, N], F32, tag="blk", name="bqp", bufs=8)
        for bb in range(NB_B):
            for nb in range(NB_N):
                nc.tensor.matmul(
                    bq_ps[:, bb * NQ:(bb + 1) * NQ],
                    vt[nb][:, bb * P128:(bb + 1) * P128],
                    pwqn[:, nb * NQ:(nb + 1) * NQ],
                    start=(nb == 0), stop=(nb == NB_N - 1),
                )
        nc.vector.tensor_copy(out=bq32[:, :], in_=bq_ps[:, 0:NB_B * NQ])

        # --- sraw-independent per-row prep, hoisted into the stage-2/3
        #     window (Act is idle until the first drains; DVE has slack) ---
        for bb in range(NB_B):
            nc.scalar.activation(out=vsq[:, :], in_=vb32[bb][:, :],
                                 func=ACTF.Square,
                                 accum_out=vn24[:, bb:bb + 1])
        vn4 = scratch.tile([P128, NB_B], F32, tag="vn4", name="vn4", bufs=2)
        nc.scalar.activation(out=vn4[:, :], in_=vn24[:, :], func=ACTF.Sqrt)
        for bb in range(NB_B):
            nc.vector.tensor_reduce(out=mdv4[:, bb:bb + 1], in_=vb32[bb][:, :],
                                    axis=AX.X, op=ALU.max,
                                    apply_absolute_value=True)
        # exp(beta) on DVE via e^b = (e^(b/64))^64 with a cubic Taylor for
        # e^(b/64): keeps the Act engine in the sqrt table set permanently.
        xq = scratch.tile([P128, NB_B], F32, tag="xq", name="xq", bufs=2)
        nc.vector.tensor_scalar_mul(out=xq[:, :], in0=beta4[:, :],
                                    scalar1=1.0 / 64.0)
        hh = scratch.tile([P128, NB_B], F32, tag="hh", name="hh", bufs=2)
        nc.vector.tensor_scalar(out=hh[:, :], in0=xq[:, :],
                                scalar1=1.0 / 6.0, scalar2=0.5,
                                op0=ALU.mult, op1=ALU.add)
        nc.vector.tensor_mul(out=hh[:, :], in0=hh[:, :], in1=xq[:, :])
        nc.vector.tensor_scalar_add(out=hh[:, :], in0=hh[:, :], scalar1=1.0)
        nc.vector.tensor_mul(out=hh[:, :], in0=hh[:, :], in1=xq[:, :])
        nc.vector.tensor_scalar_add(out=eb4[:, :], in0=hh[:, :], scalar1=1.0)
        for _sq in range(6):
            nc.vector.tensor_mul(out=eb4[:, :], in0=eb4[:, :], in1=eb4[:, :])
        # vne = max(vn, eps) * e^beta  (the non-kappa part of 1/s)
        vne4 = scratch.tile([P128, NB_B], F32, tag="vne4", name="vne4", bufs=2)
        nc.vector.tensor_scalar_max(out=vne4[:, :], in0=vn4[:, :],
                                    scalar1=1e-12)
        nc.vector.tensor_mul(out=vne4[:, :], in0=vne4[:, :], in1=eb4[:, :])

        # stage 3: fp8 triangular quadratic forms u_i = v L_i (reversed
        # nb order skips the zero blocks); drains: 5 on Act (Square+accum
        # from psum), 3 on DVE (bf16 copy + tensor_tensor_reduce)
        N_ACT_DRAIN = 6
        for bb in range(NB_B):
            u_list = []
            for i in range(NQ):
                u_list.append(mainps.tile([P128, N], F32, tag="blk",
                                          name=f"u{bb}_{i}", bufs=8))
            # nb-outer: the stationary vt[nb] slice is reused across all 8
            # constraints so the lowering elides repeated Ldweights
            for nb in range(NB_N - 1, -1, -1):
                wcols = (nb + 1) * P128
                for i in range(NQ):
                    nc.tensor.matmul(
                        u_list[i][:, 0:wcols],
                        vt[nb][:, bb * P128:(bb + 1) * P128],
                        lc8[nb][:, i * wcols:(i + 1) * wcols],
                        start=(nb == NB_N - 1), stop=(nb == 0),
                    )
            for i in range(NQ):
                u_ps = u_list[i]
                col = sraw[:, bb * NQ + i:bb * NQ + i + 1]
                if i < N_ACT_DRAIN:
                    nc.scalar.activation(
                        out=u_ps[:, :], in_=u_ps[:, :], func=ACTF.Square,
                        accum_out=col)
                else:
                    # 3-op DVE drain from individually proven instruction
                    # types: psum->bf16 copy, square via tensor_mul,
                    # add-reduce to the sraw column
                    nc.vector.tensor_copy(out=u16[:, :], in_=u_ps[:, :])
                    nc.vector.tensor_mul(out=u16b[:, :], in0=u16[:, :],
                                         in1=u16[:, :])
                    nc.vector.tensor_reduce(out=col, in_=u16b[:, :],
                                            axis=AX.X, op=ALU.add)

            # per-block finale: s4 column ready while later blocks still
            # drain, so the y scales stream instead of piling on the tail
            bbq = slice(bb * NQ, (bb + 1) * NQ)
            t8 = scratch.tile([P128, NQ], F32, tag="t8", name="t8", bufs=2)
            nc.vector.tensor_mul(out=t8[:, :], in0=bq32[:, bbq],
                                 in1=bq32[:, bbq])
            d8 = scratch.tile([P128, NQ], F32, tag="d8", name="d8", bufs=2)
            nc.vector.tensor_mul(out=d8[:, :], in0=sraw[:, bbq],
                                 in1=cm2[:, bbq])
            nc.vector.tensor_add(out=d8[:, :], in0=d8[:, :], in1=t8[:, :])
            nc.scalar.activation(out=d8[:, :], in_=d8[:, :], func=ACTF.Sqrt)
            nc.vector.tensor_add(out=d8[:, :], in0=d8[:, :], in1=bq32[:, bbq])
            nc.vector.tensor_mul(out=d8[:, :], in0=d8[:, :], in1=cinv[:, bbq])
            kap1 = scratch.tile([P128, 1], F32, tag="kap1", name="kap1", bufs=2)
            nc.vector.tensor_reduce(out=kap1[:, :], in_=d8[:, :],
                                    axis=AX.X, op=ALU.max)
            nc.vector.tensor_max(out=kap1[:, :], in0=kap1[:, :],
                                 in1=mdv4[:, bb:bb + 1])
            nc.vector.tensor_add(out=kap1[:, :], in0=kap1[:, :],
                                 in1=vne4[:, bb:bb + 1])
            nc.vector.reciprocal(out=s4[:, bb:bb + 1], in_=kap1[:, :])


        # stage 5: y matmuls weave into the stage-3 drain shadow (same pool)
        for bb in range(NB_B):
            yt = ypool.tile([P128, K], BF16, tag="yt", name="yt", bufs=3)
            for kh in range(NKH):
                yp_t = mainps.tile([P128, N], F32, tag="blk",
                                   name=f"y{bb}_{kh}", bufs=8)
                for nb in range(NB_N):
                    nc.tensor.matmul(
                        yp_t[:, :], vt[nb][:, bb * P128:(bb + 1) * P128],
                        nat[nb][:, kh * KH:(kh + 1) * KH],
                        start=(nb == 0), stop=(nb == NB_N - 1),
                    )
                nc.vector.tensor_scalar_mul(
                    out=yt[:, kh * KH:(kh + 1) * KH], in0=yp_t[:, :],
                    scalar1=s4[:, bb:bb + 1],
                )
            nc.gpsimd.dma_start(out=y_d[bb * P128:(bb + 1) * P128, :],
                                in_=yt[:, :])
        if dbg_d:
            for nm, t in [("Dsraw", sraw), ("Dbq", bq32), ("Dmdv", mdv4),
                          ("Dvn2", vn24), ("Dbeta", beta4), ("Ds4", s4)]:
                nc.sync.dma_start(out=dbg_d[nm][:, :], in_=t[:, :])
            vt0f = scratch.tile([P128, BC], F32, tag="vt0f", name="vt0f", bufs=2)
            nc.vector.tensor_copy(out=vt0f[:, :], in_=vt[0][:, :])
            nc.sync.dma_start(out=dbg_d["Dvt0"][:, :], in_=vt0f[:, :])


_NC_CACHE = {}


def _get_nc(use_f32r=True, reps=1, timing=False):
    key = (bool(use_f32r), reps, timing)
    if key not in _NC_CACHE:
        _NC_CACHE[key] = _build(use_f32r=key[0], reps=reps, timing=timing)
    return _NC_CACHE[key]


def _pow2_scale(absmax, target=200.0):
    s = 2.0 ** np.floor(np.log2(target / max(absmax, 1e-30)))
    return float(np.clip(s, 2.0 ** -10, 2.0 ** 10))


def _prepare_host(inputs):
    import ml_dtypes

    f = lambda a: np.ascontiguousarray(np.asarray(a, dtype=np.float32))
    bf = lambda a: np.ascontiguousarray(np.asarray(a).astype(ml_dtypes.bfloat16))
    f8 = lambda a: np.ascontiguousarray(
        np.asarray(a, np.float32).astype(ml_dtypes.float8_e4m3))
    x = f(inputs["x"])
    W_map = f(inputs["W_map"])
    b_map = f(inputs["b_map"])
    D = f(inputs["D"])
    NA_E = f(inputs["NA_E"])
    y1 = f(inputs["y1"])
    z0 = f(inputs["z0"])
    all_P = np.asarray(inputs["all_P"], dtype=np.float64)
    all_q = f(inputs["all_q"])
    all_r = f(inputs["all_r"])

    # structural assumptions baked into the kernel
    eye = np.eye(N, dtype=np.float32)
    assert np.allclose(D, np.concatenate([eye, -eye], axis=0), atol=1e-5), \
        "kernel assumes box constraints D == [I; -I]"
    assert np.all(b_map == 0.0), "kernel assumes zero mapper bias"

    NA64 = NA_E.astype(np.float64)
    w = (NA_E @ z0 + y1)[:, 0]                              # [K]
    w64 = w.astype(np.float64)
    Pw = all_P @ w64 + all_q[:, :, 0]                       # [NQ, K]
    cv = (
        0.5 * (all_P @ w64) @ w64
        + all_q[:, :, 0].astype(np.float64) @ w64
        + all_r[:, 0, 0]
    )                                                        # [NQ], ~ -1
    pwqn = Pw @ NA64                                         # [NQ, N]
    G = NA64.T[None, :, :] @ all_P @ NA64[None, :, :]        # [NQ, N, N]
    G = 0.5 * (G + G.transpose(0, 2, 1))

    L = np.linalg.cholesky(G)                                # lower [NQ, N, N]

    LC8 = [np.zeros((P128, NQ, (nb + 1) * P128), np.float32)
           for nb in range(NB_N)]
    svec = np.zeros(NQ)                                      # sraw_hw = svec * sraw
    v8 = 1.0  # stationary v stays bf16 (no device prescale)
    for i in range(NQ):
        sl = _pow2_scale(np.abs(L[i]).max())
        Ls = (L[i] * sl).astype(np.float32)
        svec[i] = (v8 * sl) ** 2
        for nb in range(NB_N):
            w_c = (nb + 1) * P128
            LC8[nb][:, i, :] = Ls[nb * P128:(nb + 1) * P128, 0:w_c]

    PWQN = np.zeros((P128, NB_N * NQ), np.float32)
    for nb in range(NB_N):
        PWQN[:, nb * NQ:(nb + 1) * NQ] = pwqn.T[nb * P128:(nb + 1) * P128, :]
    cm2 = np.tile((-2.0 * cv / svec).astype(np.float32), NB_B)   # [32]
    cinv = np.tile((1.0 / (-2.0 * cv)).astype(np.float32), NB_B)

    shared = dict(
        WT=bf(W_map.T), NAT=bf(NA_E.T),
        PWQN=bf(PWQN), CM2=f(cm2), CINV=f(cinv),
    )
    cvt8 = f8 if FP8ON else bf
    for nb in range(NB_N):
        shared[f"LC8_{nb}"] = cvt8(LC8[nb].reshape(P128, -1))
    in_maps = []
    for c in range(NCORES):
        m = dict(shared)
        m["XT"] = bf(x[c * BC:(c + 1) * BC, :].T)            # [IN, BC]
        in_maps.append(m)
    return in_maps, f(w)


def kernel(**inputs) -> np.ndarray:
    from concourse.bass_utils import run_bass_kernel_spmd

    in_maps, w = _prepare_host(inputs)
    nc = _get_nc()
    res = run_bass_kernel_spmd(nc, in_maps, core_ids=list(range(NCORES)))
    out = np.concatenate(
        [np.asarray(res.results[c]["Y"], dtype=np.float32)
         for c in range(NCORES)], axis=0)
    return out + w[None, :]
